# revision 1
# baseline (speedup 1.0000x reference)
"""Trainium2 Bass kernel for nn_Attention (B=2, S=2048, D=1024, H=16).

Sharding: 8 cores = 2 batches x 4 head-groups (4 heads per core).
Each core computes QKV projection for its batch restricted to its 4 heads,
full (non-causal) attention for those heads, and a partial output
projection over its 256 channels. The host sums the 4 partial outputs per
batch (the out-proj bias is fed only to head-group 0's cores).

Device dataflow (per core), matmuls in float32r (~1.5e-4 rel err, 4x the
fp32 PE throughput):
  1. x [2048,1024] -> PE-transpose -> xT [f, tok] (f32r)
  2. qk proj (natural [tok, 512]): 8 accum matmuls + K=1 ones-row bias
     -> RoPE on DVE (writes evens/odds-permuted d order) -> PE-transpose
     -> qT/kT [128, 2 head-pair chunks, 2048]
     v proj -> V [kt, 16 tiles, 4 heads, 65] with a ones column (col 64)
  3. attention per (qt-chunk, head-pair): logitsT = kT.T @ qT (row-packed
     2 heads at K=64), exp on ACT (1/8 scale fused), OT accum =
     V_aug.T @ ET over 16 kt tiles; row 64 of OT = softmax sums.
  4. divide: recip(sums) -> PE outer-product broadcast -> DVE multiply;
     odd head rows shifted to partitions 64:128 via SBUF-SBUF DMA.
  5. out-proj: y[tok,e] accum over 2 channel chunks + K=1 ones-row bias.
"""

import numpy as np

S = 2048
D = 1024
HD = 64
H_LOC = 4  # heads per core
N_CORES = 8
TT = 16  # token tiles of 128
G = 4    # token groups of 512
QC = 4   # query chunks of 512
KT = 16  # key tiles of 128

_CACHED = {}


def build_nc(repeats: int = 1, stages: str = "ABC"):
    import concourse.bass as bass
    import concourse.mybir as mybir
    from concourse import bacc
    from concourse.tile import TileContext
    from concourse.masks import make_identity

    f32 = mybir.dt.float32
    f32r = mybir.dt.float32r
    Exp = mybir.ActivationFunctionType.Exp

    nc = bacc.Bacc("TRN2", target_bir_lowering=False, debug=False,
                   num_devices=N_CORES)

    x_d = nc.dram_tensor("x", [S, D], f32, kind="ExternalInput")
    cos_d = nc.dram_tensor("cosr", [S, 32], f32, kind="ExternalInput")
    sin_d = nc.dram_tensor("sinr", [S, 32], f32, kind="ExternalInput")
    wqk_d = nc.dram_tensor("wqk", [D, 512], f32r, kind="ExternalInput")
    wv_d = nc.dram_tensor("wv", [D, 256], f32r, kind="ExternalInput")
    wout_d = nc.dram_tensor("wout", [256, D], f32r, kind="ExternalInput")
    bqk_d = nc.dram_tensor("bqk", [1, 512], f32r, kind="ExternalInput")
    bv_d = nc.dram_tensor("bv", [1, 256], f32r, kind="ExternalInput")
    bout_d = nc.dram_tensor("bout", [1, D], f32r, kind="ExternalInput")
    ones_d = nc.dram_tensor("ones", [1, 128], f32r, kind="ExternalInput")
    onescol_d = nc.dram_tensor("onescol", [128, 64], f32r,
                               kind="ExternalInput")
    y_d = nc.dram_tensor("y", [S, D], f32, kind="ExternalOutput")

    with TileContext(nc) as tc:
        with (
            tc.tile_pool(name="const", bufs=1) as cpool,
            tc.tile_pool(name="xin", bufs=1) as xpool,
            tc.tile_pool(name="xt", bufs=1) as xtpool,
            tc.tile_pool(name="qkr", bufs=1) as qkrpool,
            tc.tile_pool(name="rtmp", bufs=2) as rtpool,
            tc.tile_pool(name="big", bufs=1) as bigpool,
            tc.tile_pool(name="et", bufs=2) as etpool,
            tc.tile_pool(name="yt", bufs=2) as ypool,
            tc.tile_pool(name="sml", bufs=2) as spool,
            tc.tile_pool(name="psp", bufs=1, space="PSUM") as psp,
            tc.tile_pool(name="psl", bufs=1, space="PSUM") as psl,
            tc.tile_pool(name="pso", bufs=2, space="PSUM") as pso,
        ):
            # ---- constants / weights ----
            wqk_sb = cpool.tile([128, 8, 512], f32r)
            wv_sb = cpool.tile([128, 8, 256], f32r)
            wout_sb = cpool.tile([128, 2, D], f32r)
            cos_sb = cpool.tile([128, TT, 32], f32)
            sin_sb = cpool.tile([128, TT, 32], f32)
            bqk_sb = cpool.tile([1, 512], f32r)
            bv_sb = cpool.tile([1, 256], f32r)
            bout_sb = cpool.tile([1, D], f32r)
            ones_sb = cpool.tile([1, 128], f32r)
            onescol_sb = cpool.tile([128, 64], f32r)
            ident = cpool.tile([128, 128], f32)

            nc.sync.dma_start(wqk_sb[:], wqk_d.ap().rearrange("(i p) c -> p i c", p=128))
            nc.sync.dma_start(wv_sb[:], wv_d.ap().rearrange("(i p) c -> p i c", p=128))
            nc.sync.dma_start(wout_sb[:], wout_d.ap().rearrange("(i p) c -> p i c", p=128))
            nc.sync.dma_start(cos_sb[:], cos_d.ap().rearrange("(t p) c -> p t c", p=128))
            nc.sync.dma_start(sin_sb[:], sin_d.ap().rearrange("(t p) c -> p t c", p=128))
            nc.sync.dma_start(bqk_sb[:], bqk_d[:])
            nc.sync.dma_start(bv_sb[:], bv_d[:])
            nc.sync.dma_start(bout_sb[:], bout_d[:])
            nc.sync.dma_start(ones_sb[:], ones_d[:])
            nc.sync.dma_start(onescol_sb[:], onescol_d[:])
            make_identity(nc, ident[:])

            def body(_iv=None):
                qT = bigpool.tile([128, 2, S], f32r, tag="qT")
                kT = bigpool.tile([128, 2, S], f32r, tag="kT")
                attn = bigpool.tile([128, 2, S], f32r, tag="attn")
                v_sb = bigpool.tile([128, TT, H_LOC, 65], f32r, tag="v")
                nc.vector.tensor_copy(
                    v_sb[:, :, :, 64:65],
                    onescol_sb[:].rearrange("p (t h o) -> p t h o", h=H_LOC, o=1))

                # ================= stage A: projections =================
                import concourse.bass as bass_mod

                def bcast8(ap):
                    return bass_mod.AP(ap.tensor, ap.offset,
                                       [ap.ap[0], [0, 8], ap.ap[1]])

                for g in range(G):
                    ring = psp.tile([128, 3, 512], f32, tag="ring", name=f"ringA{g}")
                    rs = [0]

                    def rslice(n=512):
                        s = rs[0] % 3
                        rs[0] += 1
                        return ring[:, s, 0:n]

                    xts = []
                    for ti in range(4):
                        tt = g * 4 + ti
                        x_t = xpool.tile([128, D], f32, tag=f"x{ti}")
                        nc.sync.dma_start(x_t[:], x_d[tt * 128:(tt + 1) * 128, :])
                        xts.append(x_t)

                    xT_g = xtpool.tile([128, 8, 512], f32r)
                    for fc in range(8):
                        ps_x = rslice()
                        for ti in range(4):
                            nc.tensor.transpose(
                                ps_x[:, ti * 128:(ti + 1) * 128],
                                xts[ti][:, fc * 128:(fc + 1) * 128], ident[:])
                        nc.vector.tensor_copy(xT_g[:, fc, :], ps_x)

                    if stages == "A1":
                        continue
                    qkrs = []
                    for ti in range(4):
                        tt = g * 4 + ti
                        # ---- v projection ----
                        ps_v = rslice(256)
                        for fc in range(8):
                            nc.tensor.matmul(
                                ps_v, xT_g[:, fc, ti * 128:(ti + 1) * 128],
                                wv_sb[:, fc, :],
                                start=(fc == 0), stop=False)
                        nc.tensor.matmul(ps_v, ones_sb[0:1, 0:128], bv_sb[:],
                                         start=False, stop=True)
                        nc.vector.tensor_copy(
                            v_sb[:, tt, :, 0:64],
                            ps_v.rearrange("p (h d) -> p h d", h=H_LOC))

                        # ---- qk projection (natural layout) ----
                        if stages == "A2":
                            continue
                        ps_qk = rslice()
                        for fc in range(8):
                            nc.tensor.matmul(
                                ps_qk, xT_g[:, fc, ti * 128:(ti + 1) * 128],
                                wqk_sb[:, fc, :],
                                start=(fc == 0), stop=False)
                        nc.tensor.matmul(ps_qk, ones_sb[0:1, 0:128], bqk_sb[:],
                                         start=False, stop=True)

                        # ---- rope (6 batched DVE ops, step-0 cos bcast) ----
                        qk_r = qkrpool.tile([128, 512], f32, tag=f"qkr{ti}")
                        cos8 = bcast8(cos_sb[:, tt, :])
                        sin8 = bcast8(sin_sb[:, tt, :])
                        srcr = ps_qk.rearrange("p (g j two) -> p two g j",
                                               g=8, j=32, two=2)
                        dstr = qk_r[:].rearrange("p (g pm j) -> p pm g j",
                                                 pm=2, j=32)
                        ev, od = srcr[:, 0], srcr[:, 1]
                        t1 = rtpool.tile([128, 8, 32], f32, tag="t1")
                        t2 = rtpool.tile([128, 8, 32], f32, tag="t2")
                        nc.vector.tensor_mul(t1[:], od, sin8)
                        nc.vector.tensor_mul(dstr[:, 0], ev, cos8)
                        nc.vector.tensor_sub(dstr[:, 0], dstr[:, 0], t1[:])
                        nc.vector.tensor_mul(t2[:], ev, sin8)
                        nc.vector.tensor_mul(dstr[:, 1], od, cos8)
                        nc.vector.tensor_add(dstr[:, 1], dstr[:, 1], t2[:])
                        qkrs.append(qk_r)

                    # ---- transpose roped qk into qT/kT ----
                    if stages in ("A2", "A3"):
                        continue
                    for cc in range(4):
                        ps_t = rslice()
                        for ti in range(4):
                            nc.tensor.transpose(
                                ps_t[:, ti * 128:(ti + 1) * 128],
                                qkrs[ti][:, cc * 128:(cc + 1) * 128], ident[:])
                        dstbuf = qT if cc < 2 else kT
                        nc.vector.tensor_copy(
                            dstbuf[:, cc % 2, g * 512:(g + 1) * 512], ps_t)

                # ================= stage B: attention =================
                if stages == "A1":
                    for g2 in range(G):
                        pass
                if "B" not in stages:
                    # anchor stage-A results so nothing gets dead-coded
                    if stages not in ("A1", "A2"):
                        if stages != "A3":
                            nc.sync.dma_start(y_d[0:128, :],
                                              qT[:, 0, 0:1024].bitcast(f32))
                            nc.sync.dma_start(y_d[128:256, :],
                                              kT[:, 0, 0:1024].bitcast(f32))
                    if stages != "A1":
                        nc.sync.dma_start(
                            y_d[256:384, 0:260],
                            v_sb[:, 0, :, :].rearrange("p h d -> p (h d)").bitcast(f32))
                    return
                for qc in range(QC):
                    for hp in range(2):
                        O_A = pso.tile([128, 512], f32, tag="O", name="O_A")
                        O_B = pso.tile([128, 512], f32, tag="O", name="O_B")
                        lring = psl.tile([128, 3, 512], f32, tag="L")
                        ering = etpool.tile([128, 6, 512], f32r, tag="et")
                        for kt in range(KT):
                            sA, sB = (2 * kt) % 3, (2 * kt + 1) % 3
                            eA, eB = (2 * kt) % 6, (2 * kt + 1) % 6
                            nc.tensor.matmul(
                                lring[:, sA, :],
                                kT[0:64, hp, kt * 128:(kt + 1) * 128],
                                qT[0:64, hp, qc * 512:(qc + 1) * 512],
                                start=True, stop=True, tile_position=(0, 0))
                            nc.tensor.matmul(
                                lring[:, sB, :],
                                kT[64:128, hp, kt * 128:(kt + 1) * 128],
                                qT[64:128, hp, qc * 512:(qc + 1) * 512],
                                start=True, stop=True, tile_position=(64, 0))
                            nc.scalar.activation(ering[:, eA, :], lring[:, sA, :],
                                                 Exp, scale=0.125)
                            nc.scalar.activation(ering[:, eB, :], lring[:, sB, :],
                                                 Exp, scale=0.125)
                            nc.tensor.matmul(
                                O_A[0:65, :], v_sb[:, kt, 2 * hp, :], ering[:, eA, :],
                                start=(kt == 0), stop=(kt == KT - 1))
                            nc.tensor.matmul(
                                O_B[0:65, :], v_sb[:, kt, 2 * hp + 1, :], ering[:, eB, :],
                                start=(kt == 0), stop=(kt == KT - 1))
                        for (O_ps, odd) in ((O_A, 0), (O_B, 1)):
                            recip = spool.tile([1, 512], f32r, tag="rc")
                            with nc.allow_low_precision(
                                    reason="f32r reciprocal feeds f32r matmul"):
                                nc.vector.reciprocal(recip[:], O_ps[64:65, :])
                            nc.tensor.matmul(lring[0:64, odd, :], ones_sb[0:1, 0:64],
                                             recip[:], start=True, stop=True)
                            bc_sb = spool.tile([64, 512], f32, tag="bc")
                            nc.vector.tensor_copy(bc_sb[:], lring[0:64, odd, :])
                            if not odd:
                                nc.vector.tensor_mul(
                                    attn[0:64, hp, qc * 512:(qc + 1) * 512],
                                    O_ps[0:64, :], bc_sb[:])
                            else:
                                t_at = spool.tile([64, 512], f32r, tag="ta")
                                nc.vector.tensor_mul(t_at[:], O_ps[0:64, :], bc_sb[:])
                                nc.sync.dma_start(
                                    attn[64:128, hp, qc * 512:(qc + 1) * 512],
                                    t_at[:])

                    # ---- stage C: out-proj for this query chunk ----
                    if "C" not in stages:
                        nc.sync.dma_start(
                            y_d[qc * 128:(qc + 1) * 128, 0:512],
                            attn[:, 0, qc * 512:(qc + 1) * 512].bitcast(f32))
                        continue
                    ringc = psp.tile([128, 3, 512], f32, tag="ring",
                                     name=f"ringC{qc}")
                    for ti in range(4):
                        tt = qc * 4 + ti
                        y_t = ypool.tile([128, D], f32)
                        for ec in range(2):
                            ps_y = ringc[:, (ti * 2 + ec) % 3, :]
                            nc.tensor.matmul(
                                ps_y, attn[:, 0, tt * 128:(tt + 1) * 128],
                                wout_sb[:, 0, ec * 512:(ec + 1) * 512],
                                start=True, stop=False)
                            nc.tensor.matmul(
                                ps_y, attn[:, 1, tt * 128:(tt + 1) * 128],
                                wout_sb[:, 1, ec * 512:(ec + 1) * 512],
                                start=False, stop=False)
                            nc.tensor.matmul(
                                ps_y, ones_sb[0:1, 0:128],
                                bout_sb[0:1, ec * 512:(ec + 1) * 512],
                                start=False, stop=True)
                            nc.vector.tensor_copy(
                                y_t[:, ec * 512:(ec + 1) * 512], ps_y)
                        nc.sync.dma_start(y_d[tt * 128:(tt + 1) * 128, :], y_t[:])

            if repeats == 1:
                body()
            else:
                with tc.For_i(0, repeats, 1) as _i:
                    body(_i)

    nc.compile()
    return nc


def _prep_in_maps(x, rope_cos, rope_sin, W_qkv, b_qkv, W_out, b_out):
    f32 = np.float32
    W3 = np.asarray(W_qkv, dtype=f32).reshape(D, 16, 3, HD)  # [f, head, qkv, d]
    b3 = np.asarray(b_qkv, dtype=f32).reshape(16, 3, HD)
    cos_r = np.ascontiguousarray(np.asarray(rope_cos, dtype=f32))
    sin_r = np.ascontiguousarray(np.asarray(rope_sin, dtype=f32))
    ones = np.ones((1, 128), dtype=f32)
    onescol = np.ones((128, 64), dtype=f32)
    W_out = np.asarray(W_out, dtype=f32)
    b_out = np.asarray(b_out, dtype=f32)
    x = np.asarray(x, dtype=f32)

    in_maps = []
    for c in range(N_CORES):
        b, hg = divmod(c, 4)
        hs = slice(hg * H_LOC, (hg + 1) * H_LOC)
        wq = W3[:, hs, 0, :].reshape(D, 256)
        wk = W3[:, hs, 1, :].reshape(D, 256)
        wv = W3[:, hs, 2, :].reshape(D, 256)
        bq = b3[hs, 0, :].reshape(1, 256)
        bk = b3[hs, 1, :].reshape(1, 256)
        bv = b3[hs, 2, :].reshape(1, 256)
        in_maps.append({
            "x": np.ascontiguousarray(x[b]),
            "cosr": cos_r, "sinr": sin_r,
            "wqk": np.ascontiguousarray(np.concatenate([wq, wk], axis=1)),
            "wv": np.ascontiguousarray(wv),
            "wout": np.ascontiguousarray(W_out[hg * 256:(hg + 1) * 256, :]),
            "bqk": np.ascontiguousarray(np.concatenate([bq, bk], axis=1)),
            "bv": np.ascontiguousarray(bv),
            "bout": (np.ascontiguousarray(b_out.reshape(1, D)) if hg == 0
                     else np.zeros((1, D), dtype=f32)),
            "ones": ones, "onescol": onescol,
        })
    return in_maps


def kernel(x, rope_cos, rope_sin, W_qkv, b_qkv, W_out, b_out):
    from concourse.bass_utils import run_bass_kernel_spmd

    if "nc" not in _CACHED:
        _CACHED["nc"] = build_nc(1)
    nc = _CACHED["nc"]
    in_maps = _prep_in_maps(x, rope_cos, rope_sin, W_qkv, b_qkv, W_out, b_out)
    res = run_bass_kernel_spmd(nc, in_maps, list(range(N_CORES)))
    B = x.shape[0]
    out = np.zeros((B, S, D), dtype=np.float32)
    for c in range(N_CORES):
        b = c // 4
        out[b] += res.results[c]["y"]
    return out



# revision 34
# speedup vs baseline: 12184.0694x; 12184.0694x over previous
"""Trainium2 Bass kernel for nn_Attention (B=2, S=2048, D=1024, H=16).

Sharding: 8 cores = 2 batches x 4 head-groups (4 heads per core).
Each core computes QKV projection for its batch restricted to its 4 heads,
full (non-causal) attention for those heads, and a partial output
projection over its 256 channels. The host sums the 4 partial outputs per
batch.

v3 design:
 - x transposed on host -> no x PE-transposes on device.
 - one PSUM ring instance for the whole body (slice-level WAR tracking).
 - stage A per 512-token group: qk proj (natural layout) -> rope (DVE
   evens / Pool odds) -> PE transpose (f32r) with ACT PSUM drains; v proj
   drained by Pool into [keytok, head, d|1] layout with a ones column for
   softmax sums.
 - stage B software-pipelined: per key tile kt one 1024-wide exp covers
   both heads of the pair; logits for kt+1 are issued before AV(kt) so
   ACT (the bottleneck: 128 x 1038ns exps) never waits on PE. The
   out-proj (stage C) matmuls of the previous query chunk are inserted
   one-at-a-time into the per-kt PE slack; softmax divide uses a K=1
   broadcast matmul into the Y PSUM bank and partition-shifted Pool
   multiplies.
 - startup DMAs spread across engine queues (xt on SP, wqk on ACT, wv on
   Pool, cos/sin on DVE) so the first matmul starts ~1us in.
"""

import numpy as np

S = 2048
D = 1024
HD = 64
H_LOC = 4  # heads per core
N_CORES = 8
TT = 16  # token tiles of 128
G = 4    # token groups of 512
QC = 4   # query chunks of 512
KT = 16  # key tiles of 128

_CACHED = {}


def build_nc(repeats: int = 1, with_bias: bool = False):
    import concourse.bass as bass_mod
    import concourse.mybir as mybir
    from concourse import bacc
    from concourse.tile import TileContext
    f32 = mybir.dt.float32
    f32r = mybir.dt.float32r
    Exp = mybir.ActivationFunctionType.Exp

    nc = bacc.Bacc("TRN2", target_bir_lowering=False, debug=False,
                   num_devices=N_CORES)

    xt_d = nc.dram_tensor("xt", [D, S], f32r, kind="ExternalInput")
    cos_d = nc.dram_tensor("cosr", [S, 32], f32, kind="ExternalInput")
    sin_d = nc.dram_tensor("sinr", [S, 32], f32, kind="ExternalInput")
    wqk_d = nc.dram_tensor("wqk", [D, 512], f32r, kind="ExternalInput")
    wv_d = nc.dram_tensor("wv", [D, 256], f32r, kind="ExternalInput")
    wout_d = nc.dram_tensor("wout", [256, D], f32r, kind="ExternalInput")
    ones_d = nc.dram_tensor("ones", [1, 128], f32r, kind="ExternalInput")
    onescol_d = nc.dram_tensor("onescol", [128, 64], f32r,
                               kind="ExternalInput")
    ident_d = nc.dram_tensor("ident", [128, 128], f32r, kind="ExternalInput")
    if with_bias:
        bqk_d = nc.dram_tensor("bqk", [1, 512], f32r, kind="ExternalInput")
        bv_d = nc.dram_tensor("bv", [1, 256], f32r, kind="ExternalInput")
        bout_d = nc.dram_tensor("bout", [1, D], f32r, kind="ExternalInput")
    y_d = nc.dram_tensor("y", [S, D], f32, kind="ExternalOutput")

    with TileContext(nc) as tc:
        with (
            tc.tile_pool(name="const", bufs=1) as cpool,
            tc.tile_pool(name="xin", bufs=2) as xpool,
            tc.tile_pool(name="qkr", bufs=2) as qkrpool,
            tc.tile_pool(name="rtmp", bufs=2) as rtpool,
            tc.tile_pool(name="big", bufs=1) as bigpool,
            tc.tile_pool(name="et", bufs=3) as etpool,
            tc.tile_pool(name="yt", bufs=2) as ypool,
            tc.tile_pool(name="tl", bufs=1) as tailpool,
            tc.tile_pool(name="sml", bufs=1) as spool,
            tc.tile_pool(name="psl", bufs=1, space="PSUM") as psl,
            tc.tile_pool(name="pso", bufs=1, space="PSUM") as pso,
            tc.tile_pool(name="psy", bufs=2, space="PSUM") as psy,
        ):
            # ---- constants / weights (spread across engine DMA queues) ----
            wqk_sb = cpool.tile([128, 8, 512], f32r)
            wv_sb = cpool.tile([128, 8, 256], f32r)
            wout_sb = cpool.tile([128, 2, D], f32r)
            cos_sb = cpool.tile([128, TT, 32], f32)
            sin_sb = cpool.tile([128, TT, 32], f32)
            ones_sb = cpool.tile([1, 128], f32r)
            onescol_sb = cpool.tile([128, 64], f32r)
            ident = cpool.tile([128, 128], f32r)
            if with_bias:
                bqk_sb = cpool.tile([1, 512], f32r)
                bv_sb = cpool.tile([1, 256], f32r)
                bout_sb = cpool.tile([1, D], f32r)

            wqk_r = wqk_d.ap().rearrange("(i p) c -> p i c", p=128)
            for fc in range(8):
                nc.scalar.dma_start(wqk_sb[:, fc, :], wqk_r[:, fc, :])
            nc.gpsimd.dma_start(cos_sb[:], cos_d.ap().rearrange("(t p) c -> p t c", p=128))
            nc.gpsimd.dma_start(sin_sb[:], sin_d.ap().rearrange("(t p) c -> p t c", p=128))
            nc.gpsimd.dma_start(wv_sb[:], wv_d.ap().rearrange("(i p) c -> p i c", p=128))
            nc.gpsimd.dma_start(onescol_sb[:], onescol_d[:])
            nc.gpsimd.dma_start(ones_sb[:], ones_d[:])
            nc.gpsimd.dma_start(ident[:], ident_d[:])
            nc.scalar.dma_start(wout_sb[:], wout_d.ap().rearrange("(i p) c -> p i c", p=128))
            if with_bias:
                nc.gpsimd.dma_start(bqk_sb[:], bqk_d[:])
                nc.gpsimd.dma_start(bv_sb[:], bv_d[:])
                nc.gpsimd.dma_start(bout_sb[:], bout_d[:])

            def bcast8(ap):
                return bass_mod.AP(ap.tensor, ap.offset,
                                   [ap.ap[0], [0, 8], ap.ap[1]])

            def body(_iv=None):
                qT = bigpool.tile([128, 2, S], f32r, tag="qT")
                kT = bigpool.tile([128, 2, S], f32r, tag="kT")
                attn = bigpool.tile([128, 2, S], f32r, tag="attn")
                v_sb = bigpool.tile([128, TT, H_LOC, 65], f32r, tag="v")
                # PSUM dep tracking is whole-tensor: separate tensors per role
                LA = psl.tile([128, 2, 512], f32, tag="LA")
                LB = psl.tile([128, 2, 512], f32, tag="LB")
                nc.gpsimd.tensor_copy(
                    v_sb[:, :, :, 64:65],
                    onescol_sb[:].rearrange("p (t h o) -> p t h o", h=H_LOC, o=1))

                # ================= stage A: projections =================
                for g in range(G):
                    xt_g = xpool.tile([128, 8, 512], f32r, tag="xt")
                    xt_r = xt_d[:, g * 512:(g + 1) * 512].rearrange(
                        "(i p) s -> p i s", p=128)
                    if g == 0:
                        for fc in range(8):
                            nc.sync.dma_start(xt_g[:, fc, :], xt_r[:, fc, :])
                    else:
                        nc.sync.dma_start(xt_g[:], xt_r)

                    O_a = pso.tile([128, 2, 512], f32, tag="O", name=f"Oa{g}")
                    qkrs = []
                    for ti in range(4):
                        tt = g * 4 + ti
                        # ---- qk projection (natural [tok, 512]) ----
                        ps_qk = (LA, LB)[ti % 2][:, 0, :]
                        for fc in range(8):
                            nc.tensor.matmul(
                                ps_qk, xt_g[:, fc, ti * 128:(ti + 1) * 128],
                                wqk_sb[:, fc, :],
                                start=(fc == 0), stop=(not with_bias and fc == 7))
                        if with_bias:
                            nc.tensor.matmul(ps_qk, ones_sb[0:1, 0:128], bqk_sb[:],
                                             start=False, stop=True)

                        # ---- rope: ACT pre-drain (Pool can't read PSUM),
                        # then DVE evens chain, Pool odds chain, SBUF-side
                        qk_sb = tailpool.tile([128, 512], f32, tag=f"qks{ti}")
                        nc.scalar.copy(qk_sb[:], ps_qk)
                        qk_r = qkrpool.tile([128, 512], f32r, tag=f"qkr{ti}")
                        cos8 = bcast8(cos_sb[:, tt, :])
                        sin8 = bcast8(sin_sb[:, tt, :])
                        srcr = qk_sb[:].rearrange("p (g j two) -> p two g j",
                                                  g=8, j=32, two=2)
                        dstr = qk_r[:].rearrange("p (g pm j) -> p pm g j",
                                                 pm=2, j=32)
                        ev, od = srcr[:, 0], srcr[:, 1]
                        t1 = rtpool.tile([128, 8, 32], f32, tag="t1")
                        t2 = rtpool.tile([128, 8, 32], f32, tag="t2")
                        nc.vector.tensor_mul(t1[:], od, sin8)
                        nc.vector.tensor_mul(dstr[:, 0], ev, cos8)
                        nc.vector.tensor_sub(dstr[:, 0], dstr[:, 0], t1[:])
                        nc.gpsimd.tensor_mul(t2[:], ev, sin8)
                        nc.gpsimd.tensor_mul(dstr[:, 1], od, cos8)
                        nc.gpsimd.tensor_add(dstr[:, 1], dstr[:, 1], t2[:])
                        qkrs.append(qk_r)

                        # ---- v projection ----
                        ps_v = O_a[:, ti % 2, 0:256]
                        for fc in range(8):
                            nc.tensor.matmul(
                                ps_v, xt_g[:, fc, ti * 128:(ti + 1) * 128],
                                wv_sb[:, fc, :],
                                start=(fc == 0), stop=(not with_bias and fc == 7))
                        if with_bias:
                            nc.tensor.matmul(ps_v, ones_sb[0:1, 0:128], bv_sb[:],
                                             start=False, stop=True)
                        nc.vector.tensor_copy(
                            v_sb[:, tt, :, 0:64],
                            ps_v.rearrange("p (h d) -> p h d", h=H_LOC))

                    # ---- transpose roped qk into qT/kT (ACT drains) ----
                    for cc in range(4):
                        if cc == 0:
                            ps_t = LA[:, 0, :].bitcast(f32r)
                        elif cc == 1:
                            ps_t = LB[:, 0, :].bitcast(f32r)
                        elif cc == 2:
                            ps_t = O_a[:, 0, :].bitcast(f32r)
                        else:
                            ty = psy.tile([128, 512], f32, tag="Y",
                                          name=f"Ty{g}")
                            ps_t = ty[:].bitcast(f32r)
                        for ti in range(4):
                            nc.tensor.transpose(
                                ps_t[:, ti * 128:(ti + 1) * 128],
                                qkrs[ti][:, cc * 128:(cc + 1) * 128],
                                ident[:])
                        dstbuf = qT if cc < 2 else kT
                        nc.scalar.copy(
                            dstbuf[:, cc % 2, g * 512:(g + 1) * 512], ps_t)

                # ============ stage B: attention (+ C interleaved) ============
                units = [(qc, hp) for qc in range(QC) for hp in range(2)]

                def logits(qc, hp, kt):
                    tgt = (LA, LB)[kt % 2]
                    for (j, h64) in ((0, 0), (1, 64)):
                        nc.tensor.matmul(
                            tgt[:, j, :],
                            kT[h64:h64 + 64, hp, kt * 128:(kt + 1) * 128],
                            qT[h64:h64 + 64, hp, qc * 512:(qc + 1) * 512],
                            start=True, stop=True, tile_position=(h64, 0))

                def c_unit_mm(qc2, u, half, y_ps):
                    # one matmul of out-proj unit u (token tile ti, chunk ec)
                    ti, ec = divmod(u, 2)
                    tt = qc2 * 4 + ti
                    nc.tensor.matmul(
                        y_ps[:], attn[:, half, tt * 128:(tt + 1) * 128],
                        wout_sb[:, half, ec * 512:(ec + 1) * 512],
                        start=(half == 0),
                        stop=(half == 1 and not with_bias))

                def c_unit_finish(qc2, u, y_ps):
                    ti, ec = divmod(u, 2)
                    tt = qc2 * 4 + ti
                    if with_bias:
                        nc.tensor.matmul(y_ps[:], ones_sb[0:1, 0:128],
                                         bout_sb[0:1, ec * 512:(ec + 1) * 512],
                                         start=False, stop=True)
                    y_t = ypool.tile([128, 512], f32, tag="yt")
                    nc.vector.tensor_copy(y_t[:], y_ps[:])
                    nc.sync.dma_start(
                        y_d[tt * 128:(tt + 1) * 128, ec * 512:(ec + 1) * 512],
                        y_t[:])

                def divide(ui, qc, hp, O):
                    # Drain O to SBUF immediately (frees O for the next
                    # unit's AV accumulation), then run the whole softmax
                    # divide SBUF-side on DVE/Pool — zero PE involvement,
                    # off the unit-boundary critical path.
                    o_sb = spool.tile([65, 2, 512], f32r, tag="osb",
                                      name=f"osb{ui}")
                    nc.vector.tensor_copy(o_sb[:, :, :], O[0:65, :, :])
                    rec = spool.tile([1, 1024], f32r, tag="rec",
                                     name=f"rec{ui}")
                    with nc.allow_low_precision(
                            reason="f32r reciprocal feeds f32r multiply"):
                        nc.vector.reciprocal(rec[0:1, :],
                                             o_sb[64:65, :, :].rearrange(
                                                 "p a b -> p (a b)"))
                    bc = spool.tile([64, 1024], f32r, tag="bc",
                                    name=f"bc{ui}")
                    nc.gpsimd.partition_broadcast(bc[:], rec[:])
                    qs = slice(qc * 512, (qc + 1) * 512)
                    nc.gpsimd.tensor_mul(attn[0:64, hp, qs],
                                         o_sb[0:64, 0, :], bc[:, 0:512])
                    nc.gpsimd.tensor_mul(attn[64:128, hp, qs],
                                         o_sb[0:64, 1, :], bc[:, 512:1024])

                # tail half0 out-proj: ridden in the last unit's kt>=10
                # windows (they only need divide(3,0)); drained to SBUF so
                # the tail only runs the hp=1 matmuls + add-drains.
                tail_yt = {}

                def tail_half0(u):
                    ti, ec = divmod(u, 2)
                    tt = (QC - 1) * 4 + ti
                    yp = psy.tile([128, 512], f32, tag="Y", name=f"t0y{u}")
                    nc.tensor.matmul(
                        yp[:], attn[:, 0, tt * 128:(tt + 1) * 128],
                        wout_sb[:, 0, ec * 512:(ec + 1) * 512],
                        start=True, stop=True)
                    y_t = tailpool.tile([128, 512], f32, tag=f"tyt{u}")
                    nc.vector.tensor_copy(y_t[:], yp[:])
                    tail_yt[u] = y_t

                logits(0, 0, 0)  # prologue for the first unit
                for ui, (qc, hp) in enumerate(units):
                    O = pso.tile([128, 2, 512], f32, tag="O", name=f"O{ui}")
                    # pending out-proj half-matmuls for the previous chunk
                    pend = []
                    if hp == 0 and qc > 0:
                        pend = [(qc - 1, u) for u in range(4)]
                    elif hp == 1 and qc > 0:
                        pend = [(qc - 1, u) for u in range(4, 8)]
                    pend_i = 0
                    cur_y = [None]

                    for kt in range(KT):
                        ering = etpool.tile([128, 2, 512], f32r, tag="er")
                        src = (LA, LB)[kt % 2]
                        nc.scalar.activation(ering[:], src[:, :, :],
                                             Exp, scale=0.125)
                        if kt < KT - 1:
                            logits(qc, hp, kt + 1)
                        elif ui + 1 < len(units):
                            qc2, hp2 = units[ui + 1]
                            logits(qc2, hp2, 0)  # next unit's prologue
                        nc.tensor.matmul(
                            O[0:65, 0, :], v_sb[:, kt, 2 * hp, :],
                            ering[:, 0, :], start=(kt == 0), stop=(kt == KT - 1))
                        nc.tensor.matmul(
                            O[0:65, 1, :], v_sb[:, kt, 2 * hp + 1, :],
                            ering[:, 1, :], start=(kt == 0), stop=(kt == KT - 1))
                        # ride one out-proj matmul in the per-kt PE slack
                        if kt >= 2 and pend_i < 2 * len(pend):
                            qc2, u = pend[pend_i // 2]
                            half = pend_i % 2
                            if half == 0:
                                cur_y[0] = psy.tile([128, 512], f32, tag="Y",
                                                    name=f"ycu{ui}_{pend_i}")
                            c_unit_mm(qc2, u, half, cur_y[0])
                            if half == 1:
                                c_unit_finish(qc2, u, cur_y[0])
                            pend_i += 1
                        elif ui == len(units) - 1 and kt >= 10:
                            tail_half0(kt - 10)

                    divide(ui, qc, hp, O)

                # tail: hp=1 half of the last query chunk's out-proj
                for u in range(8):
                    if u not in tail_yt:
                        tail_half0(u)
                for u in range(8):
                    ti, ec = divmod(u, 2)
                    tt = (QC - 1) * 4 + ti
                    yp = psy.tile([128, 512], f32, tag="Y", name=f"t1y{u}")
                    nc.tensor.matmul(
                        yp[:], attn[:, 1, tt * 128:(tt + 1) * 128],
                        wout_sb[:, 1, ec * 512:(ec + 1) * 512],
                        start=True, stop=not with_bias)
                    if with_bias:
                        nc.tensor.matmul(yp[:], ones_sb[0:1, 0:128],
                                         bout_sb[0:1, ec * 512:(ec + 1) * 512],
                                         start=False, stop=True)
                    y_t = tail_yt[u]
                    nc.vector.tensor_add(y_t[:], y_t[:], yp[:])
                    nc.sync.dma_start(
                        y_d[tt * 128:(tt + 1) * 128, ec * 512:(ec + 1) * 512],
                        y_t[:])

            if repeats == 1:
                body()
            else:
                with tc.For_i(0, repeats, 1) as _i:
                    body(_i)

    nc.compile()
    return nc


def _prep_in_maps(x, rope_cos, rope_sin, W_qkv, b_qkv, W_out, b_out,
                  with_bias=False):
    f32 = np.float32
    W3 = np.asarray(W_qkv, dtype=f32).reshape(D, 16, 3, HD)  # [f, head, qkv, d]
    b3 = np.asarray(b_qkv, dtype=f32).reshape(16, 3, HD)
    cos_r = np.ascontiguousarray(np.asarray(rope_cos, dtype=f32))
    sin_r = np.ascontiguousarray(np.asarray(rope_sin, dtype=f32))
    ones = np.ones((1, 128), dtype=f32)
    onescol = np.ones((128, 64), dtype=f32)
    W_out = np.asarray(W_out, dtype=f32)
    b_out = np.asarray(b_out, dtype=f32)
    x = np.asarray(x, dtype=f32)

    in_maps = []
    for c in range(N_CORES):
        b, hg = divmod(c, 4)
        hs = slice(hg * H_LOC, (hg + 1) * H_LOC)
        wq = W3[:, hs, 0, :].reshape(D, 256)
        wk = W3[:, hs, 1, :].reshape(D, 256)
        wv = W3[:, hs, 2, :].reshape(D, 256)
        m = {
            "xt": np.ascontiguousarray(x[b].T),
            "cosr": cos_r, "sinr": sin_r,
            "wqk": np.ascontiguousarray(np.concatenate([wq, wk], axis=1)),
            "wv": np.ascontiguousarray(wv),
            "wout": np.ascontiguousarray(W_out[hg * 256:(hg + 1) * 256, :]),
            "ones": ones, "onescol": onescol,
            "ident": np.eye(128, dtype=f32),
        }
        if with_bias:
            bq = b3[hs, 0, :].reshape(1, 256)
            bk = b3[hs, 1, :].reshape(1, 256)
            m["bqk"] = np.ascontiguousarray(np.concatenate([bq, bk], axis=1))
            m["bv"] = np.ascontiguousarray(b3[hs, 2, :].reshape(1, 256))
            m["bout"] = (np.ascontiguousarray(b_out.reshape(1, D)) if hg == 0
                         else np.zeros((1, D), dtype=f32))
        in_maps.append(m)
    return in_maps


def kernel(x, rope_cos, rope_sin, W_qkv, b_qkv, W_out, b_out):
    from concourse.bass_utils import run_bass_kernel_spmd

    with_bias = bool(np.any(np.asarray(b_qkv)) or np.any(np.asarray(b_out)))
    key = ("nc", with_bias)
    if key not in _CACHED:
        _CACHED[key] = build_nc(1, with_bias=with_bias)
        _CACHED["nc"] = _CACHED[key]  # convenience for test harness
    nc = _CACHED[key]
    in_maps = _prep_in_maps(x, rope_cos, rope_sin, W_qkv, b_qkv, W_out, b_out,
                            with_bias=with_bias)
    res = run_bass_kernel_spmd(nc, in_maps, list(range(N_CORES)))
    B = x.shape[0]
    out = np.zeros((B, S, D), dtype=np.float32)
    for c in range(N_CORES):
        b = c // 4
        out[b] += res.results[c]["y"]
    return out


# revision 39
# speedup vs baseline: 16599.4393x; 1.3624x over previous
"""Trainium2 Bass kernel for nn_Attention (B=2, S=2048, D=1024, H=16).

Sharding: 8 cores = 2 batches x 4 head-groups (4 heads per core).
Each core computes QKV projection for its batch restricted to its 4 heads,
full (non-causal) attention for those heads, and a partial output
projection over its 256 channels. The host sums the 4 partial outputs per
batch.

v3 design:
 - x transposed on host -> no x PE-transposes on device.
 - one PSUM ring instance for the whole body (slice-level WAR tracking).
 - stage A per 512-token group: qk proj (natural layout) -> rope (DVE
   evens / Pool odds) -> PE transpose (f32r) with ACT PSUM drains; v proj
   drained by Pool into [keytok, head, d|1] layout with a ones column for
   softmax sums.
 - stage B software-pipelined: per key tile kt one 1024-wide exp covers
   both heads of the pair; logits for kt+1 are issued before AV(kt) so
   ACT (the bottleneck: 128 x 1038ns exps) never waits on PE. The
   out-proj (stage C) matmuls of the previous query chunk are inserted
   one-at-a-time into the per-kt PE slack; softmax divide uses a K=1
   broadcast matmul into the Y PSUM bank and partition-shifted Pool
   multiplies.
 - startup DMAs spread across engine queues (xt on SP, wqk on ACT, wv on
   Pool, cos/sin on DVE) so the first matmul starts ~1us in.
"""

import numpy as np

S = 2048
D = 1024
HD = 64
H_LOC = 4  # heads per core
N_CORES = 8
TT = 16  # token tiles of 128
G = 4    # token groups of 512
QC = 4   # query chunks of 512
KT = 16  # key tiles of 128

_CACHED = {}


def build_nc(repeats: int = 1, with_bias: bool = False):
    import concourse.bass as bass_mod
    import concourse.mybir as mybir
    from concourse import bacc
    from concourse.tile import TileContext
    f32 = mybir.dt.float32
    f32r = mybir.dt.float32r
    Exp = mybir.ActivationFunctionType.Exp

    nc = bacc.Bacc("TRN2", target_bir_lowering=False, debug=False,
                   num_devices=N_CORES)

    xt_d = nc.dram_tensor("xt", [D, S], f32r, kind="ExternalInput")
    cos_d = nc.dram_tensor("cosr", [S, 32], f32, kind="ExternalInput")
    sin_d = nc.dram_tensor("sinr", [S, 32], f32, kind="ExternalInput")
    wqk_d = nc.dram_tensor("wqk", [D, 512], f32r, kind="ExternalInput")
    wv_d = nc.dram_tensor("wv", [D, 256], f32r, kind="ExternalInput")
    wout_d = nc.dram_tensor("wout", [256, D], f32r, kind="ExternalInput")
    ones_d = nc.dram_tensor("ones", [1, 128], f32r, kind="ExternalInput")
    onescol_d = nc.dram_tensor("onescol", [128, 64], f32r,
                               kind="ExternalInput")
    ident_d = nc.dram_tensor("ident", [128, 128], f32r, kind="ExternalInput")
    if with_bias:
        bqk_d = nc.dram_tensor("bqk", [1, 512], f32r, kind="ExternalInput")
        bv_d = nc.dram_tensor("bv", [1, 256], f32r, kind="ExternalInput")
        bout_d = nc.dram_tensor("bout", [1, D], f32r, kind="ExternalInput")
    y_d = nc.dram_tensor("y", [S, D], f32, kind="ExternalOutput")

    with TileContext(nc) as tc:
        with (
            tc.tile_pool(name="const", bufs=1) as cpool,
            tc.tile_pool(name="xin", bufs=2) as xpool,
            tc.tile_pool(name="qkr", bufs=2) as qkrpool,
            tc.tile_pool(name="rtmp", bufs=2) as rtpool,
            tc.tile_pool(name="big", bufs=1) as bigpool,
            tc.tile_pool(name="et", bufs=3) as etpool,
            tc.tile_pool(name="yt", bufs=2) as ypool,
            tc.tile_pool(name="tl", bufs=1) as tailpool,
            tc.tile_pool(name="sml", bufs=1) as spool,
            tc.tile_pool(name="psl", bufs=1, space="PSUM") as psl,
            tc.tile_pool(name="pso", bufs=1, space="PSUM") as pso,
            tc.tile_pool(name="psy", bufs=2, space="PSUM") as psy,
        ):
            # ---- constants / weights (spread across engine DMA queues) ----
            wqk_sb = cpool.tile([128, 8, 512], f32r)
            wv_sb = cpool.tile([128, 8, 256], f32r)
            wout_sb = cpool.tile([128, 2, D], f32r)
            cos_sb = cpool.tile([128, TT, 32], f32)
            sin_sb = cpool.tile([128, TT, 32], f32)
            ones_sb = cpool.tile([1, 128], f32r)
            onescol_sb = cpool.tile([128, 64], f32r)
            ident = cpool.tile([128, 128], f32r)
            if with_bias:
                bqk_sb = cpool.tile([1, 512], f32r)
                bv_sb = cpool.tile([1, 256], f32r)
                bout_sb = cpool.tile([1, D], f32r)

            wqk_r = wqk_d.ap().rearrange("(i p) c -> p i c", p=128)
            for fc in range(8):
                nc.scalar.dma_start(wqk_sb[:, fc, :], wqk_r[:, fc, :])
            nc.gpsimd.dma_start(cos_sb[:], cos_d.ap().rearrange("(t p) c -> p t c", p=128))
            nc.gpsimd.dma_start(sin_sb[:], sin_d.ap().rearrange("(t p) c -> p t c", p=128))
            nc.gpsimd.dma_start(wv_sb[:], wv_d.ap().rearrange("(i p) c -> p i c", p=128))
            nc.gpsimd.dma_start(onescol_sb[:], onescol_d[:])
            nc.gpsimd.dma_start(ones_sb[:], ones_d[:])
            nc.gpsimd.dma_start(ident[:], ident_d[:])
            nc.scalar.dma_start(wout_sb[:], wout_d.ap().rearrange("(i p) c -> p i c", p=128))
            if with_bias:
                nc.gpsimd.dma_start(bqk_sb[:], bqk_d[:])
                nc.gpsimd.dma_start(bv_sb[:], bv_d[:])
                nc.gpsimd.dma_start(bout_sb[:], bout_d[:])

            def bcast8(ap):
                return bass_mod.AP(ap.tensor, ap.offset,
                                   [ap.ap[0], [0, 8], ap.ap[1]])

            def body(_iv=None):
                qT = bigpool.tile([128, 2, S], f32r, tag="qT")
                kT = bigpool.tile([128, 2, S], f32r, tag="kT")
                attn = bigpool.tile([128, 2, S], f32r, tag="attn")
                v_sb = bigpool.tile([128, TT, H_LOC, 65], f32r, tag="v")
                # PSUM dep tracking is whole-tensor: separate tensors per role
                LA = psl.tile([128, 2, 512], f32, tag="LA")
                LB = psl.tile([128, 2, 512], f32, tag="LB")
                nc.gpsimd.tensor_copy(
                    v_sb[:, :, :, 64:65],
                    onescol_sb[:].rearrange("p (t h o) -> p t h o", h=H_LOC, o=1))

                # ================= stage A: projections =================
                for g in range(G):
                    xt_g = xpool.tile([128, 8, 512], f32r, tag="xt")
                    xt_r = xt_d[:, g * 512:(g + 1) * 512].rearrange(
                        "(i p) s -> p i s", p=128)
                    if g == 0:
                        for fc in range(8):
                            nc.sync.dma_start(xt_g[:, fc, :], xt_r[:, fc, :])
                    else:
                        nc.sync.dma_start(xt_g[:], xt_r)

                    O_a = pso.tile([128, 2, 512], f32, tag="O", name=f"Oa{g}")
                    qkrs = []
                    for ti in range(4):
                        tt = g * 4 + ti
                        # ---- qk projection (natural [tok, 512]) ----
                        ps_qk = (LA, LB)[ti % 2][:, 0, :]
                        for fc in range(8):
                            nc.tensor.matmul(
                                ps_qk, xt_g[:, fc, ti * 128:(ti + 1) * 128],
                                wqk_sb[:, fc, :],
                                start=(fc == 0), stop=(not with_bias and fc == 7))
                        if with_bias:
                            nc.tensor.matmul(ps_qk, ones_sb[0:1, 0:128], bqk_sb[:],
                                             start=False, stop=True)

                        # ---- rope: ACT pre-drain (Pool can't read PSUM),
                        # then DVE evens chain, Pool odds chain, SBUF-side
                        qk_sb = tailpool.tile([128, 512], f32, tag=f"qks{ti}")
                        nc.scalar.copy(qk_sb[:], ps_qk)
                        qk_r = qkrpool.tile([128, 512], f32r, tag=f"qkr{ti}")
                        cos8 = bcast8(cos_sb[:, tt, :])
                        sin8 = bcast8(sin_sb[:, tt, :])
                        srcr = qk_sb[:].rearrange("p (g j two) -> p two g j",
                                                  g=8, j=32, two=2)
                        dstr = qk_r[:].rearrange("p (g pm j) -> p pm g j",
                                                 pm=2, j=32)
                        ev, od = srcr[:, 0], srcr[:, 1]
                        t1 = rtpool.tile([128, 8, 32], f32, tag="t1")
                        t2 = rtpool.tile([128, 8, 32], f32, tag="t2")
                        nc.vector.tensor_mul(t1[:], od, sin8)
                        nc.vector.tensor_mul(dstr[:, 0], ev, cos8)
                        nc.vector.tensor_sub(dstr[:, 0], dstr[:, 0], t1[:])
                        nc.gpsimd.tensor_mul(t2[:], ev, sin8)
                        nc.gpsimd.tensor_mul(dstr[:, 1], od, cos8)
                        nc.gpsimd.tensor_add(dstr[:, 1], dstr[:, 1], t2[:])
                        qkrs.append(qk_r)

                        # ---- v projection ----
                        ps_v = O_a[:, ti % 2, 0:256]
                        for fc in range(8):
                            nc.tensor.matmul(
                                ps_v, xt_g[:, fc, ti * 128:(ti + 1) * 128],
                                wv_sb[:, fc, :],
                                start=(fc == 0), stop=(not with_bias and fc == 7))
                        if with_bias:
                            nc.tensor.matmul(ps_v, ones_sb[0:1, 0:128], bv_sb[:],
                                             start=False, stop=True)
                        nc.vector.tensor_copy(
                            v_sb[:, tt, :, 0:64],
                            ps_v.rearrange("p (h d) -> p h d", h=H_LOC))

                    # ---- transpose roped qk into qT/kT (ACT drains) ----
                    for cc in range(4):
                        if cc == 0:
                            ps_t = LA[:, 0, :].bitcast(f32r)
                        elif cc == 1:
                            ps_t = LB[:, 0, :].bitcast(f32r)
                        elif cc == 2:
                            ps_t = O_a[:, 0, :].bitcast(f32r)
                        else:
                            ty = psy.tile([128, 512], f32, tag="Y",
                                          name=f"Ty{g}")
                            ps_t = ty[:].bitcast(f32r)
                        for ti in range(4):
                            nc.tensor.transpose(
                                ps_t[:, ti * 128:(ti + 1) * 128],
                                qkrs[ti][:, cc * 128:(cc + 1) * 128],
                                ident[:])
                        dstbuf = qT if cc < 2 else kT
                        nc.scalar.copy(
                            dstbuf[:, cc % 2, g * 512:(g + 1) * 512], ps_t)

                # ============ stage B: attention (+ C interleaved) ============
                units = [(qc, hp) for qc in range(QC) for hp in range(2)]

                def logits(qc, hp, kt):
                    tgt = (LA, LB)[kt % 2]
                    for (j, h64) in ((0, 0), (1, 64)):
                        nc.tensor.matmul(
                            tgt[:, j, :],
                            kT[h64:h64 + 64, hp, kt * 128:(kt + 1) * 128],
                            qT[h64:h64 + 64, hp, qc * 512:(qc + 1) * 512],
                            start=True, stop=True, tile_position=(h64, 0))

                def c_unit_mm(qc2, u, half, y_ps):
                    # one matmul of out-proj unit u (token tile ti, chunk ec)
                    ti, ec = divmod(u, 2)
                    tt = qc2 * 4 + ti
                    nc.tensor.matmul(
                        y_ps[:], attn[:, half, tt * 128:(tt + 1) * 128],
                        wout_sb[:, half, ec * 512:(ec + 1) * 512],
                        start=(half == 0),
                        stop=(half == 1 and not with_bias))

                def c_unit_finish(qc2, u, y_ps):
                    ti, ec = divmod(u, 2)
                    tt = qc2 * 4 + ti
                    if with_bias:
                        nc.tensor.matmul(y_ps[:], ones_sb[0:1, 0:128],
                                         bout_sb[0:1, ec * 512:(ec + 1) * 512],
                                         start=False, stop=True)
                    y_t = ypool.tile([128, 512], f32, tag="yt")
                    nc.vector.tensor_copy(y_t[:], y_ps[:])
                    nc.sync.dma_start(
                        y_d[tt * 128:(tt + 1) * 128, ec * 512:(ec + 1) * 512],
                        y_t[:])

                def divide1(ui, qc, hp, O):
                    # Drain O to SBUF immediately (frees O for the next
                    # unit's AV accumulation) and take the reciprocal of
                    # the sums row — all DVE, off the PE critical path.
                    o_sb = spool.tile([65, 2, 512], f32r, tag="osb",
                                      name=f"osb{ui}")
                    nc.vector.tensor_copy(o_sb[:, :, :], O[0:65, :, :])
                    rec = spool.tile([1, 1024], f32r, tag="rec",
                                     name=f"rec{ui}")
                    with nc.allow_low_precision(
                            reason="f32r reciprocal feeds f32r multiply"):
                        nc.vector.reciprocal(rec[0:1, :],
                                             o_sb[64:65, :, :].rearrange(
                                                 "p a b -> p (a b)"))
                    return (ui, qc, hp, o_sb, rec)

                def divide2(ui, qc, hp, o_sb, rec):
                    # K=1 broadcast matmuls (PE, ridden in a later window)
                    # then partition-shifted DVE multiplies into attn.
                    bce = psy.tile([128, 512], f32, tag="Y", name=f"bce{ui}")
                    nc.tensor.matmul(bce[0:64, :], ones_sb[0:1, 0:64],
                                     rec[0:1, 0:512], start=True, stop=True)
                    bco = psy.tile([128, 512], f32, tag="Y", name=f"bco{ui}")
                    nc.tensor.matmul(bco[0:64, :], ones_sb[0:1, 0:64],
                                     rec[0:1, 512:1024], start=True, stop=True)
                    qs = slice(qc * 512, (qc + 1) * 512)
                    nc.vector.tensor_mul(attn[0:64, hp, qs],
                                         o_sb[0:64, 0, :], bce[0:64, :])
                    nc.vector.tensor_mul(attn[64:128, hp, qs],
                                         o_sb[0:64, 1, :], bco[0:64, :])

                # tail half0 out-proj: ridden in the last unit's kt>=10
                # windows (they only need divide(3,0)); drained to SBUF so
                # the tail only runs the hp=1 matmuls + add-drains.
                tail_yt = {}

                def tail_half0(u):
                    ti, ec = divmod(u, 2)
                    tt = (QC - 1) * 4 + ti
                    yp = psy.tile([128, 512], f32, tag="Y", name=f"t0y{u}")
                    nc.tensor.matmul(
                        yp[:], attn[:, 0, tt * 128:(tt + 1) * 128],
                        wout_sb[:, 0, ec * 512:(ec + 1) * 512],
                        start=True, stop=True)
                    y_t = tailpool.tile([128, 512], f32, tag=f"tyt{u}")
                    nc.vector.tensor_copy(y_t[:], yp[:])
                    tail_yt[u] = y_t

                logits(0, 0, 0)  # prologue for the first unit
                pdiv = [None]
                for ui, (qc, hp) in enumerate(units):
                    O = pso.tile([128, 2, 512], f32, tag="O", name=f"O{ui}")
                    # pending out-proj half-matmuls for the previous chunk
                    pend = []
                    if hp == 0 and qc > 0:
                        pend = [(qc - 1, u) for u in range(4)]
                    elif hp == 1 and qc > 0:
                        pend = [(qc - 1, u) for u in range(4, 8)]
                    pend_i = 0
                    cur_y = [None]

                    for kt in range(KT):
                        ering = etpool.tile([128, 2, 512], f32r, tag="er")
                        src = (LA, LB)[kt % 2]
                        nc.scalar.activation(ering[:], src[:, :, :],
                                             Exp, scale=0.125)
                        if kt < KT - 1:
                            logits(qc, hp, kt + 1)
                        elif ui + 1 < len(units):
                            qc2, hp2 = units[ui + 1]
                            logits(qc2, hp2, 0)  # next unit's prologue
                        nc.tensor.matmul(
                            O[0:65, 0, :], v_sb[:, kt, 2 * hp, :],
                            ering[:, 0, :], start=(kt == 0), stop=(kt == KT - 1))
                        nc.tensor.matmul(
                            O[0:65, 1, :], v_sb[:, kt, 2 * hp + 1, :],
                            ering[:, 1, :], start=(kt == 0), stop=(kt == KT - 1))
                        if kt == 1 and pdiv[0] is not None:
                            # previous unit's broadcast+multiply rides here
                            divide2(*pdiv[0])
                            pdiv[0] = None
                        # ride one out-proj matmul in the per-kt PE slack
                        if kt >= 3 and pend_i < 2 * len(pend):
                            qc2, u = pend[pend_i // 2]
                            half = pend_i % 2
                            if half == 0:
                                cur_y[0] = psy.tile([128, 512], f32, tag="Y",
                                                    name=f"ycu{ui}_{pend_i}")
                            c_unit_mm(qc2, u, half, cur_y[0])
                            if half == 1:
                                c_unit_finish(qc2, u, cur_y[0])
                            pend_i += 1
                        elif ui == len(units) - 1 and kt >= 10:
                            tail_half0(kt - 10)

                    pdiv[0] = divide1(ui, qc, hp, O)

                # final unit's divide runs in the tail
                divide2(*pdiv[0])

                # tail: hp=1 half of the last query chunk's out-proj
                for u in range(8):
                    if u not in tail_yt:
                        tail_half0(u)
                for u in range(8):
                    ti, ec = divmod(u, 2)
                    tt = (QC - 1) * 4 + ti
                    yp = psy.tile([128, 512], f32, tag="Y", name=f"t1y{u}")
                    nc.tensor.matmul(
                        yp[:], attn[:, 1, tt * 128:(tt + 1) * 128],
                        wout_sb[:, 1, ec * 512:(ec + 1) * 512],
                        start=True, stop=not with_bias)
                    if with_bias:
                        nc.tensor.matmul(yp[:], ones_sb[0:1, 0:128],
                                         bout_sb[0:1, ec * 512:(ec + 1) * 512],
                                         start=False, stop=True)
                    y_t = tail_yt[u]
                    nc.vector.tensor_add(y_t[:], y_t[:], yp[:])
                    nc.sync.dma_start(
                        y_d[tt * 128:(tt + 1) * 128, ec * 512:(ec + 1) * 512],
                        y_t[:])

            if repeats == 1:
                body()
            else:
                with tc.For_i(0, repeats, 1) as _i:
                    body(_i)

    nc.compile()
    return nc


def _prep_in_maps(x, rope_cos, rope_sin, W_qkv, b_qkv, W_out, b_out,
                  with_bias=False):
    f32 = np.float32
    W3 = np.asarray(W_qkv, dtype=f32).reshape(D, 16, 3, HD)  # [f, head, qkv, d]
    b3 = np.asarray(b_qkv, dtype=f32).reshape(16, 3, HD)
    cos_r = np.ascontiguousarray(np.asarray(rope_cos, dtype=f32))
    sin_r = np.ascontiguousarray(np.asarray(rope_sin, dtype=f32))
    ones = np.ones((1, 128), dtype=f32)
    onescol = np.ones((128, 64), dtype=f32)
    W_out = np.asarray(W_out, dtype=f32)
    b_out = np.asarray(b_out, dtype=f32)
    x = np.asarray(x, dtype=f32)

    in_maps = []
    for c in range(N_CORES):
        b, hg = divmod(c, 4)
        hs = slice(hg * H_LOC, (hg + 1) * H_LOC)
        wq = W3[:, hs, 0, :].reshape(D, 256)
        wk = W3[:, hs, 1, :].reshape(D, 256)
        wv = W3[:, hs, 2, :].reshape(D, 256)
        m = {
            "xt": np.ascontiguousarray(x[b].T),
            "cosr": cos_r, "sinr": sin_r,
            "wqk": np.ascontiguousarray(np.concatenate([wq, wk], axis=1)),
            "wv": np.ascontiguousarray(wv),
            "wout": np.ascontiguousarray(W_out[hg * 256:(hg + 1) * 256, :]),
            "ones": ones, "onescol": onescol,
            "ident": np.eye(128, dtype=f32),
        }
        if with_bias:
            bq = b3[hs, 0, :].reshape(1, 256)
            bk = b3[hs, 1, :].reshape(1, 256)
            m["bqk"] = np.ascontiguousarray(np.concatenate([bq, bk], axis=1))
            m["bv"] = np.ascontiguousarray(b3[hs, 2, :].reshape(1, 256))
            m["bout"] = (np.ascontiguousarray(b_out.reshape(1, D)) if hg == 0
                         else np.zeros((1, D), dtype=f32))
        in_maps.append(m)
    return in_maps


def kernel(x, rope_cos, rope_sin, W_qkv, b_qkv, W_out, b_out):
    from concourse.bass_utils import run_bass_kernel_spmd

    with_bias = bool(np.any(np.asarray(b_qkv)) or np.any(np.asarray(b_out)))
    key = ("nc", with_bias)
    if key not in _CACHED:
        _CACHED[key] = build_nc(1, with_bias=with_bias)
        _CACHED["nc"] = _CACHED[key]  # convenience for test harness
    nc = _CACHED[key]
    in_maps = _prep_in_maps(x, rope_cos, rope_sin, W_qkv, b_qkv, W_out, b_out,
                            with_bias=with_bias)
    res = run_bass_kernel_spmd(nc, in_maps, list(range(N_CORES)))
    B = x.shape[0]
    out = np.zeros((B, S, D), dtype=np.float32)
    for c in range(N_CORES):
        b = c // 4
        out[b] += res.results[c]["y"]
    return out


# revision 40
# speedup vs baseline: 16953.4463x; 1.0213x over previous
"""Trainium2 Bass kernel for nn_Attention (B=2, S=2048, D=1024, H=16).

Sharding: 8 cores = 2 batches x 4 head-groups (4 heads per core).
Each core computes QKV projection for its batch restricted to its 4 heads,
full (non-causal) attention for those heads, and a partial output
projection over its 256 channels. The host sums the 4 partial outputs per
batch.

v3 design:
 - x transposed on host -> no x PE-transposes on device.
 - one PSUM ring instance for the whole body (slice-level WAR tracking).
 - stage A per 512-token group: qk proj (natural layout) -> rope (DVE
   evens / Pool odds) -> PE transpose (f32r) with ACT PSUM drains; v proj
   drained by Pool into [keytok, head, d|1] layout with a ones column for
   softmax sums.
 - stage B software-pipelined: per key tile kt one 1024-wide exp covers
   both heads of the pair; logits for kt+1 are issued before AV(kt) so
   ACT (the bottleneck: 128 x 1038ns exps) never waits on PE. The
   out-proj (stage C) matmuls of the previous query chunk are inserted
   one-at-a-time into the per-kt PE slack; softmax divide uses a K=1
   broadcast matmul into the Y PSUM bank and partition-shifted Pool
   multiplies.
 - startup DMAs spread across engine queues (xt on SP, wqk on ACT, wv on
   Pool, cos/sin on DVE) so the first matmul starts ~1us in.
"""

import numpy as np

S = 2048
D = 1024
HD = 64
H_LOC = 4  # heads per core
N_CORES = 8
TT = 16  # token tiles of 128
G = 4    # token groups of 512
QC = 4   # query chunks of 512
KT = 16  # key tiles of 128

_CACHED = {}


def build_nc(repeats: int = 1, with_bias: bool = False):
    import concourse.bass as bass_mod
    import concourse.mybir as mybir
    from concourse import bacc
    from concourse.tile import TileContext
    f32 = mybir.dt.float32
    f32r = mybir.dt.float32r
    Exp = mybir.ActivationFunctionType.Exp

    nc = bacc.Bacc("TRN2", target_bir_lowering=False, debug=False,
                   num_devices=N_CORES)

    xt_d = nc.dram_tensor("xt", [D, S], f32r, kind="ExternalInput")
    cos_d = nc.dram_tensor("cosr", [S, 32], f32, kind="ExternalInput")
    sin_d = nc.dram_tensor("sinr", [S, 32], f32, kind="ExternalInput")
    wqk_d = nc.dram_tensor("wqk", [D, 512], f32r, kind="ExternalInput")
    wv_d = nc.dram_tensor("wv", [D, 256], f32r, kind="ExternalInput")
    wout_d = nc.dram_tensor("wout", [256, D], f32r, kind="ExternalInput")
    ones_d = nc.dram_tensor("ones", [1, 128], f32r, kind="ExternalInput")
    onescol_d = nc.dram_tensor("onescol", [128, 64], f32r,
                               kind="ExternalInput")
    ident_d = nc.dram_tensor("ident", [128, 128], f32r, kind="ExternalInput")
    if with_bias:
        bqk_d = nc.dram_tensor("bqk", [1, 512], f32r, kind="ExternalInput")
        bv_d = nc.dram_tensor("bv", [1, 256], f32r, kind="ExternalInput")
        bout_d = nc.dram_tensor("bout", [1, D], f32r, kind="ExternalInput")
    y_d = nc.dram_tensor("y", [S, D], f32, kind="ExternalOutput")

    with TileContext(nc) as tc:
        with (
            tc.tile_pool(name="const", bufs=1) as cpool,
            tc.tile_pool(name="xin", bufs=2) as xpool,
            tc.tile_pool(name="qkr", bufs=2) as qkrpool,
            tc.tile_pool(name="rtmp", bufs=2) as rtpool,
            tc.tile_pool(name="big", bufs=1) as bigpool,
            tc.tile_pool(name="et", bufs=3) as etpool,
            tc.tile_pool(name="yt", bufs=2) as ypool,
            tc.tile_pool(name="tl", bufs=1) as tailpool,
            tc.tile_pool(name="sml", bufs=1) as spool,
            tc.tile_pool(name="psl", bufs=1, space="PSUM") as psl,
            tc.tile_pool(name="pso", bufs=1, space="PSUM") as pso,
            tc.tile_pool(name="psy", bufs=2, space="PSUM") as psy,
        ):
            # ---- constants / weights (spread across engine DMA queues) ----
            wqk_sb = cpool.tile([128, 8, 512], f32r)
            wv_sb = cpool.tile([128, 8, 256], f32r)
            wout_sb = cpool.tile([128, 2, D], f32r)
            cos_sb = cpool.tile([128, TT, 32], f32)
            sin_sb = cpool.tile([128, TT, 32], f32)
            ones_sb = cpool.tile([1, 128], f32r)
            onescol_sb = cpool.tile([128, 64], f32r)
            ident = cpool.tile([128, 128], f32r)
            if with_bias:
                bqk_sb = cpool.tile([1, 512], f32r)
                bv_sb = cpool.tile([1, 256], f32r)
                bout_sb = cpool.tile([1, D], f32r)

            wqk_r = wqk_d.ap().rearrange("(i p) c -> p i c", p=128)
            for fc in range(8):
                nc.scalar.dma_start(wqk_sb[:, fc, :], wqk_r[:, fc, :])
            nc.gpsimd.dma_start(cos_sb[:], cos_d.ap().rearrange("(t p) c -> p t c", p=128))
            nc.gpsimd.dma_start(sin_sb[:], sin_d.ap().rearrange("(t p) c -> p t c", p=128))
            nc.gpsimd.dma_start(wv_sb[:], wv_d.ap().rearrange("(i p) c -> p i c", p=128))
            nc.gpsimd.dma_start(onescol_sb[:], onescol_d[:])
            nc.gpsimd.dma_start(ones_sb[:], ones_d[:])
            nc.gpsimd.dma_start(ident[:], ident_d[:])
            nc.scalar.dma_start(wout_sb[:], wout_d.ap().rearrange("(i p) c -> p i c", p=128))
            if with_bias:
                nc.gpsimd.dma_start(bqk_sb[:], bqk_d[:])
                nc.gpsimd.dma_start(bv_sb[:], bv_d[:])
                nc.gpsimd.dma_start(bout_sb[:], bout_d[:])

            def bcast8(ap):
                return bass_mod.AP(ap.tensor, ap.offset,
                                   [ap.ap[0], [0, 8], ap.ap[1]])

            def body(_iv=None):
                qT = bigpool.tile([128, 2, S], f32r, tag="qT")
                kT = bigpool.tile([128, 2, S], f32r, tag="kT")
                attn = bigpool.tile([128, 2, S], f32r, tag="attn")
                v_sb = bigpool.tile([128, TT, H_LOC, 65], f32r, tag="v")
                # PSUM dep tracking is whole-tensor: separate tensors per role
                LA = psl.tile([128, 2, 512], f32, tag="LA")
                LB = psl.tile([128, 2, 512], f32, tag="LB")
                nc.vector.tensor_copy(
                    v_sb[:, :, :, 64:65],
                    onescol_sb[:].rearrange("p (t h o) -> p t h o", h=H_LOC, o=1))

                # ================= stage A: projections =================
                for g in range(G):
                    xt_g = xpool.tile([128, 8, 512], f32r, tag="xt")
                    xt_r = xt_d[:, g * 512:(g + 1) * 512].rearrange(
                        "(i p) s -> p i s", p=128)
                    if g == 0:
                        for fc in range(8):
                            nc.sync.dma_start(xt_g[:, fc, :], xt_r[:, fc, :])
                    else:
                        nc.sync.dma_start(xt_g[:], xt_r)

                    O_a = pso.tile([128, 2, 512], f32, tag="O", name=f"Oa{g}")
                    qkrs = []
                    for ti in range(4):
                        tt = g * 4 + ti
                        # ---- qk projection (natural [tok, 512]) ----
                        ps_qk = (LA, LB)[ti % 2][:, 0, :]
                        for fc in range(8):
                            nc.tensor.matmul(
                                ps_qk, xt_g[:, fc, ti * 128:(ti + 1) * 128],
                                wqk_sb[:, fc, :],
                                start=(fc == 0), stop=(not with_bias and fc == 7))
                        if with_bias:
                            nc.tensor.matmul(ps_qk, ones_sb[0:1, 0:128], bqk_sb[:],
                                             start=False, stop=True)

                        # ---- rope: ACT pre-drain (Pool can't read PSUM),
                        # then DVE evens chain, Pool odds chain, SBUF-side
                        qk_sb = tailpool.tile([128, 512], f32, tag=f"qks{ti}")
                        nc.scalar.copy(qk_sb[:], ps_qk)
                        qk_r = qkrpool.tile([128, 512], f32r, tag=f"qkr{ti}")
                        cos8 = bcast8(cos_sb[:, tt, :])
                        sin8 = bcast8(sin_sb[:, tt, :])
                        srcr = qk_sb[:].rearrange("p (g j two) -> p two g j",
                                                  g=8, j=32, two=2)
                        dstr = qk_r[:].rearrange("p (g pm j) -> p pm g j",
                                                 pm=2, j=32)
                        ev, od = srcr[:, 0], srcr[:, 1]
                        t1 = rtpool.tile([128, 8, 32], f32, tag="t1")
                        t2 = rtpool.tile([128, 8, 32], f32, tag="t2")
                        nc.vector.tensor_mul(t1[:], od, sin8)
                        nc.vector.tensor_mul(dstr[:, 0], ev, cos8)
                        nc.vector.tensor_sub(dstr[:, 0], dstr[:, 0], t1[:])
                        nc.vector.tensor_mul(t2[:], ev, sin8)
                        nc.vector.tensor_mul(dstr[:, 1], od, cos8)
                        nc.vector.tensor_add(dstr[:, 1], dstr[:, 1], t2[:])
                        qkrs.append(qk_r)

                        # ---- v projection ----
                        ps_v = O_a[:, ti % 2, 0:256]
                        for fc in range(8):
                            nc.tensor.matmul(
                                ps_v, xt_g[:, fc, ti * 128:(ti + 1) * 128],
                                wv_sb[:, fc, :],
                                start=(fc == 0), stop=(not with_bias and fc == 7))
                        if with_bias:
                            nc.tensor.matmul(ps_v, ones_sb[0:1, 0:128], bv_sb[:],
                                             start=False, stop=True)
                        nc.vector.tensor_copy(
                            v_sb[:, tt, :, 0:64],
                            ps_v.rearrange("p (h d) -> p h d", h=H_LOC))

                    # ---- transpose roped qk into qT/kT (ACT drains) ----
                    for cc in range(4):
                        if cc == 0:
                            ps_t = LA[:, 0, :].bitcast(f32r)
                        elif cc == 1:
                            ps_t = LB[:, 0, :].bitcast(f32r)
                        elif cc == 2:
                            ps_t = O_a[:, 0, :].bitcast(f32r)
                        else:
                            ty = psy.tile([128, 512], f32, tag="Y",
                                          name=f"Ty{g}")
                            ps_t = ty[:].bitcast(f32r)
                        for ti in range(4):
                            nc.tensor.transpose(
                                ps_t[:, ti * 128:(ti + 1) * 128],
                                qkrs[ti][:, cc * 128:(cc + 1) * 128],
                                ident[:])
                        dstbuf = qT if cc < 2 else kT
                        nc.scalar.copy(
                            dstbuf[:, cc % 2, g * 512:(g + 1) * 512], ps_t)

                # ============ stage B: attention (+ C interleaved) ============
                units = [(qc, hp) for qc in range(QC) for hp in range(2)]

                def logits(qc, hp, kt):
                    tgt = (LA, LB)[kt % 2]
                    for (j, h64) in ((0, 0), (1, 64)):
                        nc.tensor.matmul(
                            tgt[:, j, :],
                            kT[h64:h64 + 64, hp, kt * 128:(kt + 1) * 128],
                            qT[h64:h64 + 64, hp, qc * 512:(qc + 1) * 512],
                            start=True, stop=True, tile_position=(h64, 0))

                def c_unit_mm(qc2, u, half, y_ps):
                    # one matmul of out-proj unit u (token tile ti, chunk ec)
                    ti, ec = divmod(u, 2)
                    tt = qc2 * 4 + ti
                    nc.tensor.matmul(
                        y_ps[:], attn[:, half, tt * 128:(tt + 1) * 128],
                        wout_sb[:, half, ec * 512:(ec + 1) * 512],
                        start=(half == 0),
                        stop=(half == 1 and not with_bias))

                def c_unit_finish(qc2, u, y_ps):
                    ti, ec = divmod(u, 2)
                    tt = qc2 * 4 + ti
                    if with_bias:
                        nc.tensor.matmul(y_ps[:], ones_sb[0:1, 0:128],
                                         bout_sb[0:1, ec * 512:(ec + 1) * 512],
                                         start=False, stop=True)
                    y_t = ypool.tile([128, 512], f32, tag="yt")
                    nc.vector.tensor_copy(y_t[:], y_ps[:])
                    nc.sync.dma_start(
                        y_d[tt * 128:(tt + 1) * 128, ec * 512:(ec + 1) * 512],
                        y_t[:])

                def divide1(ui, qc, hp, O):
                    # Drain O to SBUF immediately (frees O for the next
                    # unit's AV accumulation) and take the reciprocal of
                    # the sums row — all DVE, off the PE critical path.
                    o_sb = spool.tile([65, 2, 512], f32r, tag="osb",
                                      name=f"osb{ui}")
                    nc.vector.tensor_copy(o_sb[:, :, :], O[0:65, :, :])
                    rec = spool.tile([1, 1024], f32r, tag="rec",
                                     name=f"rec{ui}")
                    with nc.allow_low_precision(
                            reason="f32r reciprocal feeds f32r multiply"):
                        nc.vector.reciprocal(rec[0:1, :],
                                             o_sb[64:65, :, :].rearrange(
                                                 "p a b -> p (a b)"))
                    return (ui, qc, hp, o_sb, rec)

                def divide2(ui, qc, hp, o_sb, rec):
                    # K=1 broadcast matmuls (PE, ridden in a later window)
                    # then partition-shifted DVE multiplies into attn.
                    bce = psy.tile([128, 512], f32, tag="Y", name=f"bce{ui}")
                    nc.tensor.matmul(bce[0:64, :], ones_sb[0:1, 0:64],
                                     rec[0:1, 0:512], start=True, stop=True)
                    bco = psy.tile([128, 512], f32, tag="Y", name=f"bco{ui}")
                    nc.tensor.matmul(bco[0:64, :], ones_sb[0:1, 0:64],
                                     rec[0:1, 512:1024], start=True, stop=True)
                    qs = slice(qc * 512, (qc + 1) * 512)
                    nc.vector.tensor_mul(attn[0:64, hp, qs],
                                         o_sb[0:64, 0, :], bce[0:64, :])
                    nc.vector.tensor_mul(attn[64:128, hp, qs],
                                         o_sb[0:64, 1, :], bco[0:64, :])

                # tail half0 out-proj: ridden in the last unit's kt>=10
                # windows (they only need divide(3,0)); drained to SBUF so
                # the tail only runs the hp=1 matmuls + add-drains.
                tail_yt = {}

                def tail_half0(u):
                    ti, ec = divmod(u, 2)
                    tt = (QC - 1) * 4 + ti
                    yp = psy.tile([128, 512], f32, tag="Y", name=f"t0y{u}")
                    nc.tensor.matmul(
                        yp[:], attn[:, 0, tt * 128:(tt + 1) * 128],
                        wout_sb[:, 0, ec * 512:(ec + 1) * 512],
                        start=True, stop=True)
                    y_t = tailpool.tile([128, 512], f32, tag=f"tyt{u}")
                    nc.vector.tensor_copy(y_t[:], yp[:])
                    tail_yt[u] = y_t

                logits(0, 0, 0)  # prologue for the first unit
                pdiv = [None]
                for ui, (qc, hp) in enumerate(units):
                    O = pso.tile([128, 2, 512], f32, tag="O", name=f"O{ui}")
                    # pending out-proj half-matmuls for the previous chunk
                    pend = []
                    if hp == 0 and qc > 0:
                        pend = [(qc - 1, u) for u in range(4)]
                    elif hp == 1 and qc > 0:
                        pend = [(qc - 1, u) for u in range(4, 8)]
                    pend_i = 0
                    cur_y = [None]

                    for kt in range(KT):
                        ering = etpool.tile([128, 2, 512], f32r, tag="er")
                        src = (LA, LB)[kt % 2]
                        nc.scalar.activation(ering[:], src[:, :, :],
                                             Exp, scale=0.125)
                        if kt < KT - 1:
                            logits(qc, hp, kt + 1)
                        elif ui + 1 < len(units):
                            qc2, hp2 = units[ui + 1]
                            logits(qc2, hp2, 0)  # next unit's prologue
                        nc.tensor.matmul(
                            O[0:65, 0, :], v_sb[:, kt, 2 * hp, :],
                            ering[:, 0, :], start=(kt == 0), stop=(kt == KT - 1))
                        nc.tensor.matmul(
                            O[0:65, 1, :], v_sb[:, kt, 2 * hp + 1, :],
                            ering[:, 1, :], start=(kt == 0), stop=(kt == KT - 1))
                        if kt == 1 and pdiv[0] is not None:
                            # previous unit's broadcast+multiply rides here
                            divide2(*pdiv[0])
                            pdiv[0] = None
                        # ride one out-proj matmul in the per-kt PE slack
                        if kt >= 3 and pend_i < 2 * len(pend):
                            qc2, u = pend[pend_i // 2]
                            half = pend_i % 2
                            if half == 0:
                                cur_y[0] = psy.tile([128, 512], f32, tag="Y",
                                                    name=f"ycu{ui}_{pend_i}")
                            c_unit_mm(qc2, u, half, cur_y[0])
                            if half == 1:
                                c_unit_finish(qc2, u, cur_y[0])
                            pend_i += 1
                        elif ui == len(units) - 1 and kt >= 10:
                            tail_half0(kt - 10)

                    pdiv[0] = divide1(ui, qc, hp, O)

                # final unit's divide runs in the tail
                divide2(*pdiv[0])

                # tail: hp=1 half of the last query chunk's out-proj
                for u in range(8):
                    if u not in tail_yt:
                        tail_half0(u)
                for u in range(8):
                    ti, ec = divmod(u, 2)
                    tt = (QC - 1) * 4 + ti
                    yp = psy.tile([128, 512], f32, tag="Y", name=f"t1y{u}")
                    nc.tensor.matmul(
                        yp[:], attn[:, 1, tt * 128:(tt + 1) * 128],
                        wout_sb[:, 1, ec * 512:(ec + 1) * 512],
                        start=True, stop=not with_bias)
                    if with_bias:
                        nc.tensor.matmul(yp[:], ones_sb[0:1, 0:128],
                                         bout_sb[0:1, ec * 512:(ec + 1) * 512],
                                         start=False, stop=True)
                    y_t = tail_yt[u]
                    nc.vector.tensor_add(y_t[:], y_t[:], yp[:])
                    nc.sync.dma_start(
                        y_d[tt * 128:(tt + 1) * 128, ec * 512:(ec + 1) * 512],
                        y_t[:])

            if repeats == 1:
                body()
            else:
                with tc.For_i(0, repeats, 1) as _i:
                    body(_i)

    nc.compile()
    return nc


def _prep_in_maps(x, rope_cos, rope_sin, W_qkv, b_qkv, W_out, b_out,
                  with_bias=False):
    f32 = np.float32
    W3 = np.asarray(W_qkv, dtype=f32).reshape(D, 16, 3, HD)  # [f, head, qkv, d]
    b3 = np.asarray(b_qkv, dtype=f32).reshape(16, 3, HD)
    cos_r = np.ascontiguousarray(np.asarray(rope_cos, dtype=f32))
    sin_r = np.ascontiguousarray(np.asarray(rope_sin, dtype=f32))
    ones = np.ones((1, 128), dtype=f32)
    onescol = np.ones((128, 64), dtype=f32)
    W_out = np.asarray(W_out, dtype=f32)
    b_out = np.asarray(b_out, dtype=f32)
    x = np.asarray(x, dtype=f32)

    in_maps = []
    for c in range(N_CORES):
        b, hg = divmod(c, 4)
        hs = slice(hg * H_LOC, (hg + 1) * H_LOC)
        wq = W3[:, hs, 0, :].reshape(D, 256)
        wk = W3[:, hs, 1, :].reshape(D, 256)
        wv = W3[:, hs, 2, :].reshape(D, 256)
        m = {
            "xt": np.ascontiguousarray(x[b].T),
            "cosr": cos_r, "sinr": sin_r,
            "wqk": np.ascontiguousarray(np.concatenate([wq, wk], axis=1)),
            "wv": np.ascontiguousarray(wv),
            "wout": np.ascontiguousarray(W_out[hg * 256:(hg + 1) * 256, :]),
            "ones": ones, "onescol": onescol,
            "ident": np.eye(128, dtype=f32),
        }
        if with_bias:
            bq = b3[hs, 0, :].reshape(1, 256)
            bk = b3[hs, 1, :].reshape(1, 256)
            m["bqk"] = np.ascontiguousarray(np.concatenate([bq, bk], axis=1))
            m["bv"] = np.ascontiguousarray(b3[hs, 2, :].reshape(1, 256))
            m["bout"] = (np.ascontiguousarray(b_out.reshape(1, D)) if hg == 0
                         else np.zeros((1, D), dtype=f32))
        in_maps.append(m)
    return in_maps


def kernel(x, rope_cos, rope_sin, W_qkv, b_qkv, W_out, b_out):
    from concourse.bass_utils import run_bass_kernel_spmd

    with_bias = bool(np.any(np.asarray(b_qkv)) or np.any(np.asarray(b_out)))
    key = ("nc", with_bias)
    if key not in _CACHED:
        _CACHED[key] = build_nc(1, with_bias=with_bias)
        _CACHED["nc"] = _CACHED[key]  # convenience for test harness
    nc = _CACHED[key]
    in_maps = _prep_in_maps(x, rope_cos, rope_sin, W_qkv, b_qkv, W_out, b_out,
                            with_bias=with_bias)
    res = run_bass_kernel_spmd(nc, in_maps, list(range(N_CORES)))
    B = x.shape[0]
    out = np.zeros((B, S, D), dtype=np.float32)
    for c in range(N_CORES):
        b = c // 4
        out[b] += res.results[c]["y"]
    return out


# revision 42
# speedup vs baseline: 19807.5600x; 1.1684x over previous
"""Trainium2 Bass kernel for nn_Attention (B=2, S=2048, D=1024, H=16).

Sharding: 8 cores = 2 batches x 4 head-groups (4 heads per core).
Each core computes QKV projection for its batch restricted to its 4 heads,
full (non-causal) attention for those heads, and a partial output
projection over its 256 channels. The host sums the 4 partial outputs per
batch.

v3 design:
 - x transposed on host -> no x PE-transposes on device.
 - one PSUM ring instance for the whole body (slice-level WAR tracking).
 - stage A per 512-token group: qk proj (natural layout) -> rope (DVE
   evens / Pool odds) -> PE transpose (f32r) with ACT PSUM drains; v proj
   drained by Pool into [keytok, head, d|1] layout with a ones column for
   softmax sums.
 - stage B software-pipelined: per key tile kt one 1024-wide exp covers
   both heads of the pair; logits for kt+1 are issued before AV(kt) so
   ACT (the bottleneck: 128 x 1038ns exps) never waits on PE. The
   out-proj (stage C) matmuls of the previous query chunk are inserted
   one-at-a-time into the per-kt PE slack; softmax divide uses a K=1
   broadcast matmul into the Y PSUM bank and partition-shifted Pool
   multiplies.
 - startup DMAs spread across engine queues (xt on SP, wqk on ACT, wv on
   Pool, cos/sin on DVE) so the first matmul starts ~1us in.
"""

import numpy as np

S = 2048
D = 1024
HD = 64
H_LOC = 4  # heads per core
N_CORES = 8
TT = 16  # token tiles of 128
G = 4    # token groups of 512
QC = 4   # query chunks of 512
KT = 16  # key tiles of 128

_CACHED = {}


def build_nc(repeats: int = 1, with_bias: bool = False):
    import concourse.bass as bass_mod
    import concourse.mybir as mybir
    from concourse import bacc
    from concourse.tile import TileContext
    f32 = mybir.dt.float32
    f32r = mybir.dt.float32r
    Exp = mybir.ActivationFunctionType.Exp

    nc = bacc.Bacc("TRN2", target_bir_lowering=False, debug=False,
                   num_devices=N_CORES)

    xt_d = nc.dram_tensor("xt", [D, S], f32r, kind="ExternalInput")
    cos_d = nc.dram_tensor("cosr", [S, 32], f32, kind="ExternalInput")
    sin_d = nc.dram_tensor("sinr", [S, 32], f32, kind="ExternalInput")
    wqk_d = nc.dram_tensor("wqk", [D, 512], f32r, kind="ExternalInput")
    wv_d = nc.dram_tensor("wv", [D, 256], f32r, kind="ExternalInput")
    wout_d = nc.dram_tensor("wout", [256, D], f32r, kind="ExternalInput")
    ones_d = nc.dram_tensor("ones", [1, 128], f32r, kind="ExternalInput")
    onescol_d = nc.dram_tensor("onescol", [128, 64], f32r,
                               kind="ExternalInput")
    ident_d = nc.dram_tensor("ident", [128, 128], f32r, kind="ExternalInput")
    if with_bias:
        bqk_d = nc.dram_tensor("bqk", [1, 512], f32r, kind="ExternalInput")
        bv_d = nc.dram_tensor("bv", [1, 256], f32r, kind="ExternalInput")
        bout_d = nc.dram_tensor("bout", [1, D], f32r, kind="ExternalInput")
    y_d = nc.dram_tensor("y", [S, D], f32, kind="ExternalOutput")

    with TileContext(nc) as tc:
        with (
            tc.tile_pool(name="const", bufs=1) as cpool,
            tc.tile_pool(name="xin", bufs=2) as xpool,
            tc.tile_pool(name="qkr", bufs=2) as qkrpool,
            tc.tile_pool(name="rtmp", bufs=2) as rtpool,
            tc.tile_pool(name="big", bufs=1) as bigpool,
            tc.tile_pool(name="et", bufs=3) as etpool,
            tc.tile_pool(name="yt", bufs=2) as ypool,
            tc.tile_pool(name="tl", bufs=1) as tailpool,
            tc.tile_pool(name="sml", bufs=1) as spool,
            tc.tile_pool(name="psl", bufs=1, space="PSUM") as psl,
            tc.tile_pool(name="pso", bufs=1, space="PSUM") as pso,
            tc.tile_pool(name="psy", bufs=2, space="PSUM") as psy,
        ):
            # ---- constants / weights (spread across engine DMA queues) ----
            wqk_sb = cpool.tile([128, 8, 512], f32r)
            wv_sb = cpool.tile([128, 8, 256], f32r)
            wout_sb = cpool.tile([128, 2, D], f32r)
            cos_sb = cpool.tile([128, TT, 32], f32)
            sin_sb = cpool.tile([128, TT, 32], f32)
            ones_sb = cpool.tile([1, 128], f32r)
            onescol_sb = cpool.tile([128, 64], f32r)
            ident = cpool.tile([128, 128], f32r)
            if with_bias:
                bqk_sb = cpool.tile([1, 512], f32r)
                bv_sb = cpool.tile([1, 256], f32r)
                bout_sb = cpool.tile([1, D], f32r)

            wqk_r = wqk_d.ap().rearrange("(i p) c -> p i c", p=128)
            for fc in range(8):
                nc.scalar.dma_start(wqk_sb[:, fc, :], wqk_r[:, fc, :])
            nc.gpsimd.dma_start(cos_sb[:], cos_d.ap().rearrange("(t p) c -> p t c", p=128))
            nc.gpsimd.dma_start(sin_sb[:], sin_d.ap().rearrange("(t p) c -> p t c", p=128))
            nc.gpsimd.dma_start(wv_sb[:], wv_d.ap().rearrange("(i p) c -> p i c", p=128))
            nc.gpsimd.dma_start(onescol_sb[:], onescol_d[:])
            nc.gpsimd.dma_start(ones_sb[:], ones_d[:])
            nc.gpsimd.dma_start(ident[:], ident_d[:])
            nc.scalar.dma_start(wout_sb[:], wout_d.ap().rearrange("(i p) c -> p i c", p=128))
            if with_bias:
                nc.gpsimd.dma_start(bqk_sb[:], bqk_d[:])
                nc.gpsimd.dma_start(bv_sb[:], bv_d[:])
                nc.gpsimd.dma_start(bout_sb[:], bout_d[:])

            def bcast8t(ap):
                # [p, t, j] -> [p, t, (bcast 8), j]
                return bass_mod.AP(ap.tensor, ap.offset,
                                   [ap.ap[0], ap.ap[1], [0, 8], ap.ap[2]])

            def body(_iv=None):
                qT = bigpool.tile([128, 2, S], f32r, tag="qT")
                kT = bigpool.tile([128, 2, S], f32r, tag="kT")
                attn = bigpool.tile([128, 2, S], f32r, tag="attn")
                v_sb = bigpool.tile([128, TT, H_LOC, 65], f32r, tag="v")
                # PSUM dep tracking is whole-tensor: separate tensors per role
                LA = psl.tile([128, 2, 512], f32, tag="LA")
                LB = psl.tile([128, 2, 512], f32, tag="LB")
                nc.vector.tensor_copy(
                    v_sb[:, :, :, 64:65],
                    onescol_sb[:].rearrange("p (t h o) -> p t h o", h=H_LOC, o=1))

                # ================= stage A: projections =================
                for g in range(G):
                    xt_g = xpool.tile([128, 8, 512], f32r, tag="xt")
                    xt_r = xt_d[:, g * 512:(g + 1) * 512].rearrange(
                        "(i p) s -> p i s", p=128)
                    if g == 0:
                        for fc in range(8):
                            nc.sync.dma_start(xt_g[:, fc, :], xt_r[:, fc, :])
                    else:
                        nc.sync.dma_start(xt_g[:], xt_r)

                    O_a = pso.tile([128, 2, 512], f32, tag="O", name=f"Oa{g}")
                    qk_sb = tailpool.tile([128, 4, 512], f32, tag="qks",
                                          name=f"qks{g}")
                    qk_r = qkrpool.tile([128, 4, 512], f32r, tag="qkr")
                    for ti in range(4):
                        tt = g * 4 + ti
                        # ---- qk projection (natural [tok, 512]) ----
                        ps_qk = (LA, LB)[ti // 2][:, ti % 2, :]
                        for fc in range(8):
                            nc.tensor.matmul(
                                ps_qk, xt_g[:, fc, ti * 128:(ti + 1) * 128],
                                wqk_sb[:, fc, :],
                                start=(fc == 0), stop=(not with_bias and fc == 7))
                        if with_bias:
                            nc.tensor.matmul(ps_qk, ones_sb[0:1, 0:128], bqk_sb[:],
                                             start=False, stop=True)

                        # ---- v projection ----
                        ps_v = O_a[:, ti % 2, 0:256]
                        for fc in range(8):
                            nc.tensor.matmul(
                                ps_v, xt_g[:, fc, ti * 128:(ti + 1) * 128],
                                wv_sb[:, fc, :],
                                start=(fc == 0), stop=(not with_bias and fc == 7))
                        if with_bias:
                            nc.tensor.matmul(ps_v, ones_sb[0:1, 0:128], bv_sb[:],
                                             start=False, stop=True)

                        if ti % 2 == 1:
                            # paired drains + paired rope for (ti-1, ti)
                            pr = ti // 2  # pair index 0/1
                            ts = slice(2 * pr, 2 * pr + 2)
                            nc.scalar.copy(qk_sb[:, ts, :],
                                           (LA, LB)[pr][:, :, :])
                            nc.vector.tensor_copy(
                                v_sb[:, tt - 1:tt + 1, :, 0:64],
                                O_a[:, :, 0:256].rearrange(
                                    "p t (h d) -> p t h d", h=H_LOC))
                            # rope on [128, 2, 8, 32] (512 elems per op)
                            cosp = bcast8t(cos_sb[:, tt - 1:tt + 1, :])
                            sinp = bcast8t(sin_sb[:, tt - 1:tt + 1, :])
                            srcr = qk_sb[:, ts, :].rearrange(
                                "p t (g j two) -> p two t g j", g=8, j=32)
                            dstr = qk_r[:, ts, :].rearrange(
                                "p t (g pm j) -> p pm t g j", pm=2, j=32)
                            ev, od = srcr[:, 0], srcr[:, 1]
                            t1 = rtpool.tile([128, 2, 8, 32], f32, tag="t1")
                            t2 = rtpool.tile([128, 2, 8, 32], f32, tag="t2")
                            nc.vector.tensor_mul(t1[:], od, sinp)
                            nc.vector.tensor_mul(dstr[:, 0], ev, cosp)
                            nc.vector.tensor_sub(dstr[:, 0], dstr[:, 0], t1[:])
                            nc.vector.tensor_mul(t2[:], ev, sinp)
                            nc.vector.tensor_mul(dstr[:, 1], od, cosp)
                            nc.vector.tensor_add(dstr[:, 1], dstr[:, 1], t2[:])

                    # ---- transpose roped qk into qT/kT (paired ACT drains) ----
                    for cc in range(4):
                        tgt = (LA, LB)[cc // 2]
                        ps_t = tgt[:, cc % 2, :].bitcast(f32r)
                        for ti in range(4):
                            nc.tensor.transpose(
                                ps_t[:, ti * 128:(ti + 1) * 128],
                                qk_r[:, ti, cc * 128:(cc + 1) * 128],
                                ident[:])
                        if cc % 2 == 1:
                            dstbuf = qT if cc < 2 else kT
                            nc.scalar.copy(
                                dstbuf[:, :, g * 512:(g + 1) * 512],
                                tgt[:, :, :].bitcast(f32r))

                # ============ stage B: attention (+ C interleaved) ============
                units = [(qc, hp) for qc in range(QC) for hp in range(2)]

                def logits(qc, hp, kt):
                    tgt = (LA, LB)[kt % 2]
                    for (j, h64) in ((0, 0), (1, 64)):
                        nc.tensor.matmul(
                            tgt[:, j, :],
                            kT[h64:h64 + 64, hp, kt * 128:(kt + 1) * 128],
                            qT[h64:h64 + 64, hp, qc * 512:(qc + 1) * 512],
                            start=True, stop=True, tile_position=(h64, 0))

                def c_unit_mm(qc2, u, half, y_ps):
                    # one matmul of out-proj unit u (token tile ti, chunk ec)
                    ti, ec = divmod(u, 2)
                    tt = qc2 * 4 + ti
                    nc.tensor.matmul(
                        y_ps[:], attn[:, half, tt * 128:(tt + 1) * 128],
                        wout_sb[:, half, ec * 512:(ec + 1) * 512],
                        start=(half == 0),
                        stop=(half == 1 and not with_bias))

                def c_unit_finish(qc2, u, y_ps):
                    ti, ec = divmod(u, 2)
                    tt = qc2 * 4 + ti
                    if with_bias:
                        nc.tensor.matmul(y_ps[:], ones_sb[0:1, 0:128],
                                         bout_sb[0:1, ec * 512:(ec + 1) * 512],
                                         start=False, stop=True)
                    y_t = ypool.tile([128, 512], f32, tag="yt")
                    nc.vector.tensor_copy(y_t[:], y_ps[:])
                    nc.sync.dma_start(
                        y_d[tt * 128:(tt + 1) * 128, ec * 512:(ec + 1) * 512],
                        y_t[:])

                def divide1(ui, qc, hp, O):
                    # Drain O to SBUF immediately (frees O for the next
                    # unit's AV accumulation) and take the reciprocal of
                    # the sums row — all DVE, off the PE critical path.
                    o_sb = spool.tile([65, 2, 512], f32r, tag="osb",
                                      name=f"osb{ui}")
                    nc.vector.tensor_copy(o_sb[:, :, :], O[0:65, :, :])
                    rec = spool.tile([1, 1024], f32r, tag="rec",
                                     name=f"rec{ui}")
                    with nc.allow_low_precision(
                            reason="f32r reciprocal feeds f32r multiply"):
                        nc.vector.reciprocal(rec[0:1, :],
                                             o_sb[64:65, :, :].rearrange(
                                                 "p a b -> p (a b)"))
                    return (ui, qc, hp, o_sb, rec)

                def divide2(ui, qc, hp, o_sb, rec):
                    # K=1 broadcast matmuls (PE, ridden in a later window)
                    # then partition-shifted DVE multiplies into attn.
                    bce = psy.tile([128, 512], f32, tag="Y", name=f"bce{ui}")
                    nc.tensor.matmul(bce[0:64, :], ones_sb[0:1, 0:64],
                                     rec[0:1, 0:512], start=True, stop=True)
                    bco = psy.tile([128, 512], f32, tag="Y", name=f"bco{ui}")
                    nc.tensor.matmul(bco[0:64, :], ones_sb[0:1, 0:64],
                                     rec[0:1, 512:1024], start=True, stop=True)
                    qs = slice(qc * 512, (qc + 1) * 512)
                    nc.vector.tensor_mul(attn[0:64, hp, qs],
                                         o_sb[0:64, 0, :], bce[0:64, :])
                    nc.vector.tensor_mul(attn[64:128, hp, qs],
                                         o_sb[0:64, 1, :], bco[0:64, :])

                # tail half0 out-proj: ridden in the last unit's kt>=10
                # windows (they only need divide(3,0)); drained to SBUF so
                # the tail only runs the hp=1 matmuls + add-drains.
                tail_yt = {}

                def tail_half0(u):
                    ti, ec = divmod(u, 2)
                    tt = (QC - 1) * 4 + ti
                    yp = psy.tile([128, 512], f32, tag="Y", name=f"t0y{u}")
                    nc.tensor.matmul(
                        yp[:], attn[:, 0, tt * 128:(tt + 1) * 128],
                        wout_sb[:, 0, ec * 512:(ec + 1) * 512],
                        start=True, stop=True)
                    y_t = tailpool.tile([128, 512], f32, tag=f"tyt{u}")
                    nc.vector.tensor_copy(y_t[:], yp[:])
                    tail_yt[u] = y_t

                logits(0, 0, 0)  # prologue for the first unit
                pdiv = [None]
                for ui, (qc, hp) in enumerate(units):
                    O = pso.tile([128, 2, 512], f32, tag="O", name=f"O{ui}")
                    # pending out-proj half-matmuls for the previous chunk
                    pend = []
                    if hp == 0 and qc > 0:
                        pend = [(qc - 1, u) for u in range(4)]
                    elif hp == 1 and qc > 0:
                        pend = [(qc - 1, u) for u in range(4, 8)]
                    pend_i = 0
                    cur_y = [None]

                    for kt in range(KT):
                        ering = etpool.tile([128, 2, 512], f32r, tag="er")
                        src = (LA, LB)[kt % 2]
                        nc.scalar.activation(ering[:], src[:, :, :],
                                             Exp, scale=0.125)
                        if kt < KT - 1:
                            logits(qc, hp, kt + 1)
                        elif ui + 1 < len(units):
                            qc2, hp2 = units[ui + 1]
                            logits(qc2, hp2, 0)  # next unit's prologue
                        nc.tensor.matmul(
                            O[0:65, 0, :], v_sb[:, kt, 2 * hp, :],
                            ering[:, 0, :], start=(kt == 0), stop=(kt == KT - 1))
                        nc.tensor.matmul(
                            O[0:65, 1, :], v_sb[:, kt, 2 * hp + 1, :],
                            ering[:, 1, :], start=(kt == 0), stop=(kt == KT - 1))
                        if kt == 1 and pdiv[0] is not None:
                            # previous unit's broadcast+multiply rides here
                            divide2(*pdiv[0])
                            pdiv[0] = None
                        # ride one out-proj matmul in the per-kt PE slack
                        if kt >= 3 and pend_i < 2 * len(pend):
                            qc2, u = pend[pend_i // 2]
                            half = pend_i % 2
                            if half == 0:
                                cur_y[0] = psy.tile([128, 512], f32, tag="Y",
                                                    name=f"ycu{ui}_{pend_i}")
                            c_unit_mm(qc2, u, half, cur_y[0])
                            if half == 1:
                                c_unit_finish(qc2, u, cur_y[0])
                            pend_i += 1
                        elif ui == len(units) - 1 and kt >= 10:
                            tail_half0(kt - 10)

                    pdiv[0] = divide1(ui, qc, hp, O)

                # final unit's divide runs in the tail
                divide2(*pdiv[0])

                # tail: hp=1 half of the last query chunk's out-proj
                for u in range(8):
                    if u not in tail_yt:
                        tail_half0(u)
                for u in range(8):
                    ti, ec = divmod(u, 2)
                    tt = (QC - 1) * 4 + ti
                    yp = psy.tile([128, 512], f32, tag="Y", name=f"t1y{u}")
                    nc.tensor.matmul(
                        yp[:], attn[:, 1, tt * 128:(tt + 1) * 128],
                        wout_sb[:, 1, ec * 512:(ec + 1) * 512],
                        start=True, stop=not with_bias)
                    if with_bias:
                        nc.tensor.matmul(yp[:], ones_sb[0:1, 0:128],
                                         bout_sb[0:1, ec * 512:(ec + 1) * 512],
                                         start=False, stop=True)
                    y_t = tail_yt[u]
                    nc.vector.tensor_add(y_t[:], y_t[:], yp[:])
                    nc.sync.dma_start(
                        y_d[tt * 128:(tt + 1) * 128, ec * 512:(ec + 1) * 512],
                        y_t[:])

            if repeats == 1:
                body()
            else:
                with tc.For_i(0, repeats, 1) as _i:
                    body(_i)

    nc.compile()
    return nc


def _prep_in_maps(x, rope_cos, rope_sin, W_qkv, b_qkv, W_out, b_out,
                  with_bias=False):
    f32 = np.float32
    W3 = np.asarray(W_qkv, dtype=f32).reshape(D, 16, 3, HD)  # [f, head, qkv, d]
    b3 = np.asarray(b_qkv, dtype=f32).reshape(16, 3, HD)
    cos_r = np.ascontiguousarray(np.asarray(rope_cos, dtype=f32))
    sin_r = np.ascontiguousarray(np.asarray(rope_sin, dtype=f32))
    ones = np.ones((1, 128), dtype=f32)
    onescol = np.ones((128, 64), dtype=f32)
    W_out = np.asarray(W_out, dtype=f32)
    b_out = np.asarray(b_out, dtype=f32)
    x = np.asarray(x, dtype=f32)

    in_maps = []
    for c in range(N_CORES):
        b, hg = divmod(c, 4)
        hs = slice(hg * H_LOC, (hg + 1) * H_LOC)
        wq = W3[:, hs, 0, :].reshape(D, 256)
        wk = W3[:, hs, 1, :].reshape(D, 256)
        wv = W3[:, hs, 2, :].reshape(D, 256)
        m = {
            "xt": np.ascontiguousarray(x[b].T),
            "cosr": cos_r, "sinr": sin_r,
            "wqk": np.ascontiguousarray(np.concatenate([wq, wk], axis=1)),
            "wv": np.ascontiguousarray(wv),
            "wout": np.ascontiguousarray(W_out[hg * 256:(hg + 1) * 256, :]),
            "ones": ones, "onescol": onescol,
            "ident": np.eye(128, dtype=f32),
        }
        if with_bias:
            bq = b3[hs, 0, :].reshape(1, 256)
            bk = b3[hs, 1, :].reshape(1, 256)
            m["bqk"] = np.ascontiguousarray(np.concatenate([bq, bk], axis=1))
            m["bv"] = np.ascontiguousarray(b3[hs, 2, :].reshape(1, 256))
            m["bout"] = (np.ascontiguousarray(b_out.reshape(1, D)) if hg == 0
                         else np.zeros((1, D), dtype=f32))
        in_maps.append(m)
    return in_maps


def kernel(x, rope_cos, rope_sin, W_qkv, b_qkv, W_out, b_out):
    from concourse.bass_utils import run_bass_kernel_spmd

    with_bias = bool(np.any(np.asarray(b_qkv)) or np.any(np.asarray(b_out)))
    key = ("nc", with_bias)
    if key not in _CACHED:
        _CACHED[key] = build_nc(1, with_bias=with_bias)
        _CACHED["nc"] = _CACHED[key]  # convenience for test harness
    nc = _CACHED[key]
    in_maps = _prep_in_maps(x, rope_cos, rope_sin, W_qkv, b_qkv, W_out, b_out,
                            with_bias=with_bias)
    res = run_bass_kernel_spmd(nc, in_maps, list(range(N_CORES)))
    B = x.shape[0]
    out = np.zeros((B, S, D), dtype=np.float32)
    for c in range(N_CORES):
        b = c // 4
        out[b] += res.results[c]["y"]
    return out
